# revision 58
# baseline (speedup 1.0000x reference)
"""Trainium2 Bass kernel for a sparse-causal-attention BasicTransformerBlock.

Sharding: pure data-parallel over the 8 video frames (batch=1, f=8) -- one
frame per NeuronCore, zero collectives.  Each core receives its own frame
plus frame 0 and the previous frame (the sparse-causal KV sources) and
recomputes LN1 + K/V projections for those locally.

v2 design (vs the first working version):
  - every matmul operand is bf16 (fp8 for the FFN) -> half-cost LDWEIGHTS,
    half weight DMA traffic; PSUM accumulation stays fp32.
  - softmax normalization: denominator rows are stacked into a [8, q] tile,
    one batched DVE reciprocal per q-half, broadcast across partitions with
    a K=8 one-hot PE matmul (no DRAM round-trips, no per-head reciprocal).
  - encoder-side attn2 work (enc transpose, K2/V2 projections) hoisted to
    the program start to fill the PE while hs3 streams in.
  - FFN runs fp8e4 DoubleRow matmuls (2 contraction rows/cycle).  To dodge
    fp8 denormals the LN3 output is pre-scaled by 1/8 and w1 by 8 (exact
    cancellation); the geglu product is scaled by 4 and w2 by 16, undone
    with a single *1/64 in the final residual add.
"""

import numpy as np

P = 128
S = 1024          # tokens per frame
D = 640
H = 8
DH = 80
KV = 2 * S        # sparse-causal kv tokens (first frame + prev frame)
ENC = 77
CROSS = 768
FFI = 2560
NQT = S // P      # 8 token tiles
NKT = KV // P     # 16 kv token tiles
ND = D // P       # 5 dim tiles
NE = CROSS // P   # 6 encoder-dim tiles
NM1 = 2 * FFI // P  # 40 ff_w1 out tiles
NP1 = 3             # ff_w1 contraction pairs (640 -> padded 768 = 3*256)
NP2 = FFI // 256    # 10 ff_w2 contraction pairs
SCALE = DH ** -0.5
EPS = 1e-5
F = 8

_PROGRAM_CACHE = {}


def _build_program(flags):
    import concourse.bass as bass
    import concourse.tile as tile
    from concourse import bacc, mybir
    from concourse.masks import make_identity

    f32 = mybir.dt.float32
    f32r = mybir.dt.float32r
    bf16 = mybir.dt.bfloat16
    fp8 = mybir.dt.float8e4
    AF = mybir.ActivationFunctionType
    Alu = mybir.AluOpType
    DR = mybir.MatmulPerfMode.DoubleRow
    PSUM = bass.MemorySpace.PSUM

    (has_qb1, has_kb1, has_vb1, has_ob1, has_q2b, has_ob2, has_b1,
     has_fb2) = flags

    nc = bacc.Bacc(None, target_bir_lowering=False)

    hs3_d = nc.dram_tensor("hs3", [3 * S, D], f32, kind="ExternalInput")[:]
    enc_d = nc.dram_tensor("enc", [ENC, CROSS], f32, kind="ExternalInput")[:]
    wq1_d = nc.dram_tensor("wq1", [D, D], bf16, kind="ExternalInput")[:]
    wk1_d = nc.dram_tensor("wk1", [D, D], bf16, kind="ExternalInput")[:]
    wv1_d = nc.dram_tensor("wv1", [D, D], bf16, kind="ExternalInput")[:]
    o1p_d = nc.dram_tensor("o1p", [H, DH, D], bf16, kind="ExternalInput")[:]
    wq2_d = nc.dram_tensor("wq2", [D, D], bf16, kind="ExternalInput")[:]
    wk2_d = nc.dram_tensor("wk2", [CROSS, D], f32r, kind="ExternalInput")[:]
    wv2_d = nc.dram_tensor("wv2", [CROSS, D], f32r, kind="ExternalInput")[:]
    o2p_d = nc.dram_tensor("o2p", [H, DH, D], bf16, kind="ExternalInput")[:]
    w1p_d = nc.dram_tensor("w1p8", [NM1, P, NP1, 2, P], fp8,
                           kind="ExternalInput")[:]
    w2p_d = nc.dram_tensor("w2p8", [P, NP2, 2, D], fp8,
                           kind="ExternalInput")[:]
    out_d = nc.dram_tensor("out", [S, D], f32, kind="ExternalOutput")[:]

    b1p_d = qb1_d = kb1_d = vb1_d = q2b_d = None
    ob1_d = ob2_d = fb2_d = None
    if has_b1:
        b1p_d = nc.dram_tensor("b1p", [P, NM1], f32, kind="ExternalInput")[:]
    if has_qb1:
        qb1_d = nc.dram_tensor("qb1", [DH, H], f32, kind="ExternalInput")[:]
    if has_kb1:
        kb1_d = nc.dram_tensor("kb1", [DH, H], f32, kind="ExternalInput")[:]
    if has_vb1:
        vb1_d = nc.dram_tensor("vb1", [DH, H], f32, kind="ExternalInput")[:]
    if has_q2b:
        q2b_d = nc.dram_tensor("q2b", [DH, H], f32, kind="ExternalInput")[:]
    if has_ob1:
        ob1_d = nc.dram_tensor("ob1", [P, D], f32, kind="ExternalInput")[:]
    if has_ob2:
        ob2_d = nc.dram_tensor("ob2", [P, D], f32, kind="ExternalInput")[:]
    if has_fb2:
        fb2_d = nc.dram_tensor("fb2", [P, D], f32, kind="ExternalInput")[:]

    with tile.TileContext(nc) as tc:
        # ---------- whole-kernel constants ----------
        const = tc.alloc_tile_pool(name="const", bufs=1)
        idb = const.tile([P, P], bf16, name="idb")
        make_identity(nc, idb)
        idf = const.tile([P, P], f32, name="idf")
        make_identity(nc, idf)
        epst = const.tile([P, 1], f32, name="epst")
        nc.vector.memset(epst[:], EPS)
        ones_f32 = const.tile([P, DH], f32, name="ones_f32")
        nc.vector.memset(ones_f32[:], 1.0)
        onest = const.tile([1, DH], f32r, name="onest")
        nc.vector.tensor_copy(onest[:], ones_f32[0:1, :])
        # one-hot rows for the denominator broadcast: sel[:, h, :] has row h
        # all-ones (out = recip[h] broadcast over DH partitions).  Built with
        # affine_select (block identity): keep 0 where (x - h) != 0 else 1.
        ones_sel = const.tile([8, H, DH], f32r, name="ones_sel")
        ones_scr = const.tile([8, H, DH], f32, name="ones_scr")
        nc.gpsimd.memset(ones_scr[:], 0.0)
        nc.gpsimd.affine_select(
            out=ones_scr[:], in_=ones_scr[:],
            compare_op=Alu.not_equal, fill=1.0, base=0,
            pattern=[[-1, H], [0, DH]], channel_multiplier=1)
        nc.vector.tensor_copy(ones_sel[:], ones_scr[:])
        bias_tiles = {}
        if has_b1:
            t = const.tile([P, NM1], f32, name="b1pt")
            nc.sync.dma_start(out=t[:], in_=b1p_d)
            bias_tiles["b1p"] = t
        for nm, dref in (("qb1", qb1_d), ("kb1", kb1_d), ("vb1", vb1_d),
                         ("q2b", q2b_d)):
            if dref is not None:
                t = const.tile([DH, H], f32, name=nm + "t")
                nc.sync.dma_start(out=t[:], in_=dref)
                bias_tiles[nm] = t
        for nm, dref in (("ob1", ob1_d), ("ob2", ob2_d), ("fb2", fb2_d)):
            if dref is not None:
                t = const.tile([P, D], f32, name=nm + "t")
                nc.sync.dma_start(out=t[:], in_=dref)
                bias_tiles[nm] = t

        stats = tc.alloc_tile_pool(name="stats", bufs=6)
        io = tc.alloc_tile_pool(name="io", bufs=3)
        xop = tc.alloc_tile_pool(name="xop", bufs=3)
        h2p = tc.alloc_tile_pool(name="h2p", bufs=1)
        h2 = h2p.tile([P, NQT, D], f32, name="h2")

        def ln_block(xin, xT_dst, pt_pool, rstd_mult=None):
            """LayerNorm (scale/bias folded into consuming weights) and
            per-128-block PE transpose of the bf16 output.

            xin [128, 640] f32 sbuf; xT_dst(kt) -> [128,128] bf16 dst AP."""
            st = stats.tile([P, 2, 6], f32, name="st", tag="st")
            nc.vector.bn_stats(st[:, 0, :], xin[:, 0:320])
            nc.vector.bn_stats(st[:, 1, :], xin[:, 320:640])
            mv = stats.tile([P, 2], f32, name="mv", tag="mv")
            nc.vector.bn_aggr(mv[:], st[:])
            rstd = stats.tile([P, 1], f32, name="rstd", tag="rstd")
            nc.scalar.activation(rstd[:], mv[:, 1:2], AF.Sqrt, bias=epst[:])
            nc.vector.reciprocal(rstd[:], rstd[:])
            if rstd_mult is not None:
                nc.vector.tensor_scalar(
                    out=rstd[:], in0=rstd[:], scalar1=rstd_mult,
                    scalar2=None, op0=Alu.mult, op1=Alu.bypass)
            mb = stats.tile([P, 1], f32, name="mb", tag="mb")
            nc.vector.tensor_scalar(
                out=mb[:], in0=mv[:, 0:1], scalar1=rstd[:], scalar2=-1.0,
                op0=Alu.mult, op1=Alu.mult)
            xo = xop.tile([P, D], bf16, name="xo", tag="xo")
            nc.scalar.activation(xo[:], xin, AF.Identity,
                                 scale=rstd[:], bias=mb[:])
            for kt in range(ND):
                ptile = pt_pool.tile([P, P], bf16, name="ptile", tag="pt")
                nc.tensor.transpose(ptile[:], xo[:, kt * P:(kt + 1) * P],
                                    idb[:])
                if kt % 2 == 0:
                    nc.scalar.copy(out=xT_dst(kt), in_=ptile[:])
                else:
                    nc.vector.tensor_copy(xT_dst(kt), ptile[:])

        # ================= encoder-side attn2 prep (fills PE early) ======
        k2Tp = tc.alloc_tile_pool(name="k2Tp", bufs=1)
        k2T = k2Tp.tile([P, H, ENC], bf16, name="k2T")
        v2pp = tc.alloc_tile_pool(name="v2pp", bufs=1)
        v2p = v2pp.tile([ENC, H * (DH + 1)], f32r, name="v2p")
        encTp = tc.alloc_tile_pool(name="encTp", bufs=1)
        # fp32r matmuls need an even moving-dim size; pad 77 -> 78 zeros
        encT = encTp.tile([P, NE, ENC + 1], f32r, name="encT")
        zcol = encTp.tile([P, NE], f32, name="zcol")
        nc.vector.memset(zcol[:], 0.0)
        nc.vector.tensor_copy(
            encT[:, :, ENC:ENC + 1],
            zcol[:].rearrange("p (a b) -> p a b", b=1))
        encp = tc.alloc_tile_pool(name="encp", bufs=1)
        enc_sb = encp.tile([ENC, CROSS], f32, name="enc_sb")
        nc.sync.dma_start(out=enc_sb[:], in_=enc_d)
        ptpe = tc.alloc_tile_pool(name="ptpe", bufs=2, space=PSUM)
        for kt in range(NE):
            ptile = ptpe.tile([P, P], f32, name="ptileE", tag="pt")
            nc.tensor.transpose(ptile[0:P, 0:ENC],
                                enc_sb[:, kt * P:(kt + 1) * P],
                                idf[0:ENC, 0:ENC])
            nc.vector.tensor_copy(encT[:, kt, 0:ENC], ptile[0:P, 0:ENC])

        wk2p = tc.alloc_tile_pool(name="wk2p", bufs=6)
        pje = tc.alloc_tile_pool(name="pje", bufs=2, space=PSUM)
        wk2 = []
        for kt in range(NE):
            w = wk2p.tile([P, D], f32r, name=f"wk2{kt}", tag="w")
            nc.sync.dma_start(out=w[:], in_=wk2_d[kt * P:(kt + 1) * P, :])
            wk2.append(w)
        for h in range(H):
            pk2 = pje.tile([P, P], f32, name="pk2", tag="pj")
            for kt in range(NE):
                nc.tensor.matmul(pk2[0:DH, 0:ENC + 1],
                                 wk2[kt][:, h * DH:(h + 1) * DH],
                                 encT[:, kt, :],
                                 start=(kt == 0), stop=(kt == NE - 1))
            nc.scalar.copy(out=k2T[0:DH, h, :], in_=pk2[0:DH, 0:ENC])
        wv2 = []
        for kt in range(NE):
            w = wk2p.tile([P, D], f32r, name=f"wv2{kt}", tag="w")
            nc.sync.dma_start(out=w[:], in_=wv2_d[kt * P:(kt + 1) * P, :])
            wv2.append(w)
        pv20 = pje.tile([P, 320], f32, name="pv20", tag="pj")
        pv21 = pje.tile([P, 320], f32, name="pv21", tag="pj")
        for kt in range(NE):
            nc.tensor.matmul(pv20[0:ENC, :], encT[:, kt, 0:ENC],
                             wv2[kt][:, 0:320],
                             start=(kt == 0), stop=(kt == NE - 1))
            nc.tensor.matmul(pv21[0:ENC, :], encT[:, kt, 0:ENC],
                             wv2[kt][:, 320:640],
                             start=(kt == 0), stop=(kt == NE - 1))
        v2sl = v2p[:].rearrange("p (a b) -> p a b", b=DH + 1)
        nc.vector.tensor_copy(v2sl[:, 0:4, 0:DH],
                              pv20[0:ENC, :].rearrange("p (a b) -> p a b",
                                                       b=DH))
        nc.vector.tensor_copy(v2sl[:, 4:8, 0:DH],
                              pv21[0:ENC, :].rearrange("p (a b) -> p a b",
                                                       b=DH))
        nc.vector.tensor_copy(
            v2sl[:, :, DH:DH + 1],
            ones_f32[0:ENC, 0:H].rearrange("p (a b) -> p a b", b=1))
        wk2p.release()
        pje.release()
        ptpe.release()
        encp.release()
        encTp.release()

        # ================= LN1 + QKV projections =================
        w2sbp = tc.alloc_tile_pool(name="w2sbp", bufs=1)
        w2sb = w2sbp.tile([P, NP2, 2, D], fp8, name="w2sb")
        nc.sync.dma_start(out=w2sb[:], in_=w2p_d)
        kTp = tc.alloc_tile_pool(name="kTp", bufs=1)
        kT = kTp.tile([P, H, KV], bf16, name="kT")
        vpp = tc.alloc_tile_pool(name="vpp", bufs=1)
        vp = vpp.tile([P, NKT, H * (DH + 1)], bf16, name="vp")
        qTp = tc.alloc_tile_pool(name="qTp", bufs=1)
        qT = qTp.tile([P, H, S], bf16, name="qT")

        xowp = tc.alloc_tile_pool(name="xowp", bufs=1)
        xowT = xowp.tile([P, ND, S], bf16, name="xowT")
        xkvp = tc.alloc_tile_pool(name="xkvp", bufs=1)
        xkvT = xkvp.tile([P, ND, KV], bf16, name="xkvT")
        ptp = tc.alloc_tile_pool(name="ptp", bufs=2, space=PSUM)
        pjp = tc.alloc_tile_pool(name="pjp", bufs=2, space=PSUM)
        pvp = tc.alloc_tile_pool(name="pvp", bufs=2, space=PSUM)
        wkvp = tc.alloc_tile_pool(name="wkvp", bufs=6)

        # own frame (rows 0:1024) -> LN -> xowT
        for t in range(NQT):
            xt = io.tile([P, D], f32, name="xt", tag="io")
            nc.sync.dma_start(out=xt[:], in_=hs3_d[t * P:(t + 1) * P, :])
            ln_block(xt[:],
                     lambda kt, t=t: xowT[:, kt, t * P:(t + 1) * P], ptp)
        # kv frames (rows 1024:3072) -> LN -> xkvT
        for t in range(NKT):
            xt = io.tile([P, D], f32, name="xt", tag="io")
            nc.sync.dma_start(out=xt[:],
                              in_=hs3_d[(NQT + t) * P:(NQT + t + 1) * P, :])
            ln_block(xt[:],
                     lambda kt, t=t: xkvT[:, kt, t * P:(t + 1) * P], ptp)

        # Q projection (per head; moving = xowT chunks)
        wq = []
        for kt in range(ND):
            w = wkvp.tile([P, D], bf16, name=f"wq{kt}", tag="w")
            nc.sync.dma_start(out=w[:], in_=wq1_d[kt * P:(kt + 1) * P, :])
            wq.append(w)
        for h in range(H):
            pq = pjp.tile([P, 1024], f32, name="pq", tag="pj")
            for c in range(2):
                for kt in range(ND):
                    nc.tensor.matmul(
                        pq[0:DH, c * 512:(c + 1) * 512],
                        wq[kt][:, h * DH:(h + 1) * DH],
                        xowT[:, kt, c * 512:(c + 1) * 512],
                        start=(kt == 0), stop=(kt == ND - 1))
            if has_qb1:
                nc.vector.tensor_scalar_add(
                    pq[0:DH, :], pq[0:DH, :], bias_tiles["qb1"][:, h:h + 1])
            if h % 2 == 0:
                nc.scalar.copy(out=qT[0:DH, h, :], in_=pq[0:DH, :])
            else:
                nc.vector.tensor_copy(qT[0:DH, h, :], pq[0:DH, :])

        # V projection (stationary = xkvT tiles, moving = wv)
        wv = []
        for kt in range(ND):
            w = wkvp.tile([P, D], bf16, name=f"wv{kt}", tag="w")
            nc.sync.dma_start(out=w[:], in_=wv1_d[kt * P:(kt + 1) * P, :])
            wv.append(w)
        for m in range(NKT):
            pv0 = pvp.tile([P, 320], f32, name="pv0", tag="pv")
            pv1 = pvp.tile([P, 320], f32, name="pv1", tag="pv")
            for kt in range(ND):
                nc.tensor.matmul(pv0[:], xkvT[:, kt, m * P:(m + 1) * P],
                                 wv[kt][:, 0:320],
                                 start=(kt == 0), stop=(kt == ND - 1))
                nc.tensor.matmul(pv1[:], xkvT[:, kt, m * P:(m + 1) * P],
                                 wv[kt][:, 320:640],
                                 start=(kt == 0), stop=(kt == ND - 1))
            vsl = vp[:, m, :].rearrange("p (a b) -> p a b", b=DH + 1)
            nc.vector.tensor_copy(
                vsl[:, 0:4, 0:DH], pv0[:].rearrange("p (a b) -> p a b", b=DH))
            nc.vector.tensor_copy(
                vsl[:, 4:8, 0:DH], pv1[:].rearrange("p (a b) -> p a b", b=DH))
            nc.vector.memset(vsl[:, :, DH:DH + 1], 1.0)

        wk = []
        for kt in range(ND):
            w = wkvp.tile([P, D], bf16, name=f"wk{kt}", tag="w")
            nc.sync.dma_start(out=w[:], in_=wk1_d[kt * P:(kt + 1) * P, :])
            wk.append(w)
        pvp.release()
        pjp.release()
        ptp.release()

        # ======== attn1: K-proj fused into the per-head attention loop ====
        # Head h's attention starts as soon as its own K tiles are done,
        # instead of waiting for all 8 heads' projections.
        outTnp = tc.alloc_tile_pool(name="outTnp", bufs=1)
        outTn = outTnp.tile([P, H, S], bf16, name="outTn")
        o1pp = tc.alloc_tile_pool(name="o1pp", bufs=1)
        o1pt = o1pp.tile([P, H, D], bf16, name="o1pt")
        for h in range(H):
            nc.sync.dma_start(out=o1pt[0:DH, h, :], in_=o1p_d[h])
        expp = tc.alloc_tile_pool(name="expp", bufs=4)
        dstkp = tc.alloc_tile_pool(name="dstkp", bufs=4)
        psp = tc.alloc_tile_pool(name="psp", bufs=2, space=PSUM)
        pavp = tc.alloc_tile_pool(name="pavp", bufs=2, space=PSUM)
        popb = tc.alloc_tile_pool(name="popb", bufs=2, space=PSUM)

        dstacks = [dstkp.tile([8, 512], bf16, name=f"dstack{qc}", tag="dst")
                   for qc in range(2)]
        rstacks = [dstkp.tile([8, 512], f32r, name=f"rstack{qc}", tag="rst")
                   for qc in range(2)]
        for h in range(H):
            # K projection for this head.  PSUM comes from the popb pool
            # (idle until the normalization tail) so K-proj copies never
            # starve the scores pipeline of ps slots.
            for c4 in range(4):
                pk = popb.tile([P, 512], f32, name="pk", tag="popb")
                for kt in range(ND):
                    nc.tensor.matmul(
                        pk[0:DH, :],
                        wk[kt][:, h * DH:(h + 1) * DH],
                        xkvT[:, kt, c4 * 512:(c4 + 1) * 512],
                        start=(kt == 0), stop=(kt == ND - 1))
                if has_kb1:
                    nc.vector.tensor_scalar_add(
                        pk[0:DH, :], pk[0:DH, :],
                        bias_tiles["kb1"][:, h:h + 1])
                nc.vector.tensor_copy(
                    kT[0:DH, h, c4 * 512:(c4 + 1) * 512], pk[0:DH, :])
            for qc in range(2):
                pav = pavp.tile([P, 512], f32, name="pav", tag="pav")
                for kp in range(NKT // 2):
                    ps = psp.tile([P, 1024], f32, name="ps", tag="ps")
                    for half in range(2):
                        kvt = 2 * kp + half
                        nc.tensor.matmul(
                            ps[:, half * 512:(half + 1) * 512],
                            kT[0:DH, h, kvt * P:(kvt + 1) * P],
                            qT[0:DH, h, qc * 512:(qc + 1) * 512],
                            start=True, stop=True)
                    ex = expp.tile([P, 1024], bf16, name="ex", tag="exp")
                    nc.scalar.activation(ex[:], ps[:], AF.Exp, scale=SCALE)
                    for half in range(2):
                        kvt = 2 * kp + half
                        nc.tensor.matmul(
                            pav[0:DH + 1, :],
                            vp[:, kvt, h * (DH + 1):(h + 1) * (DH + 1)],
                            ex[:, half * 512:(half + 1) * 512],
                            start=(kvt == 0), stop=(kvt == NKT - 1))
                qs = slice(qc * 512, (qc + 1) * 512)
                nc.vector.tensor_copy(outTn[0:DH + 1, h, qs],
                                      pav[0:DH + 1, :])
                nc.gpsimd.dma_start(out=dstacks[qc][h:h + 1, :],
                                    in_=outTn[DH:DH + 1, h, qs])
        for qc in range(2):
            with nc.allow_low_precision(reason="f32r softmax denom recip"):
                nc.vector.reciprocal(rstacks[qc][:], dstacks[qc][:])
            for h in range(H):
                qs = slice(qc * 512, (qc + 1) * 512)
                pb = popb.tile([P, 512], f32, name="pb", tag="popb")
                nc.tensor.matmul(pb[0:DH, :], ones_sel[0:8, h, :],
                                 rstacks[qc][:], start=True, stop=True)
                nc.vector.tensor_mul(outTn[0:DH, h, qs],
                                     outTn[0:DH, h, qs], pb[0:DH, :])
                if has_vb1:
                    nc.vector.tensor_scalar_add(
                        outTn[0:DH, h, qc * 512:(qc + 1) * 512],
                        outTn[0:DH, h, qc * 512:(qc + 1) * 512],
                        bias_tiles["vb1"][:, h:h + 1])
        # o1 projection + residual
        for t in range(NQT):
            hres = io.tile([P, D], f32, name="hres", tag="io")
            nc.sync.dma_start(out=hres[:], in_=hs3_d[t * P:(t + 1) * P, :])
            po0 = popb.tile([P, 512], f32, name="po0", tag="popb")
            po1 = popb.tile([P, 512], f32, name="po1", tag="popb")
            for h in range(H):
                nc.tensor.matmul(po0[:, 0:320],
                                 outTn[0:DH, h, t * P:(t + 1) * P],
                                 o1pt[0:DH, h, 0:320],
                                 start=(h == 0), stop=(h == H - 1))
                nc.tensor.matmul(po1[:, 0:320],
                                 outTn[0:DH, h, t * P:(t + 1) * P],
                                 o1pt[0:DH, h, 320:640],
                                 start=(h == 0), stop=(h == H - 1))
            nc.vector.tensor_add(h2[:, t, 0:320], po0[:, 0:320],
                                 hres[:, 0:320])
            nc.vector.tensor_add(h2[:, t, 320:640], po1[:, 0:320],
                                 hres[:, 320:640])
            if has_ob1:
                nc.vector.tensor_add(h2[:, t, :], h2[:, t, :],
                                     bias_tiles["ob1"][:])
        popb.release()
        pavp.release()
        psp.release()
        dstkp.release()
        expp.release()
        o1pp.release()
        outTnp.release()
        wkvp.release()
        xkvp.release()
        xowp.release()
        qTp.release()
        vpp.release()
        kTp.release()

        # ================= attn2: cross attention =================
        h3p = tc.alloc_tile_pool(name="h3p", bufs=1)
        h3 = h3p.tile([P, NQT, D], f32, name="h3")
        q2Tp = tc.alloc_tile_pool(name="q2Tp", bufs=1)
        q2T = q2Tp.tile([P, H, S], bf16, name="q2T")
        x2p = tc.alloc_tile_pool(name="x2p", bufs=1)
        x2T = x2p.tile([P, ND, S], bf16, name="x2T")
        ptp2 = tc.alloc_tile_pool(name="ptp2", bufs=2, space=PSUM)
        pjp2 = tc.alloc_tile_pool(name="pjp2", bufs=1, space=PSUM)
        pav2p = tc.alloc_tile_pool(name="pav2p", bufs=2, space=PSUM)
        for t in range(NQT):
            ln_block(h2[:, t, :],
                     lambda kt, t=t: x2T[:, kt, t * P:(t + 1) * P], ptp2)

        wq2p = tc.alloc_tile_pool(name="wq2p", bufs=5)
        wq2 = []
        for kt in range(ND):
            w = wq2p.tile([P, D], bf16, name=f"wq2{kt}", tag="w")
            nc.sync.dma_start(out=w[:], in_=wq2_d[kt * P:(kt + 1) * P, :])
            wq2.append(w)
        for h in range(H):
            pq = pjp2.tile([P, 1024], f32, name="pq2", tag="pj2")
            for c in range(2):
                for kt in range(ND):
                    nc.tensor.matmul(
                        pq[0:DH, c * 512:(c + 1) * 512],
                        wq2[kt][:, h * DH:(h + 1) * DH],
                        x2T[:, kt, c * 512:(c + 1) * 512],
                        start=(kt == 0), stop=(kt == ND - 1))
            if has_q2b:
                nc.vector.tensor_scalar_add(
                    pq[0:DH, :], pq[0:DH, :], bias_tiles["q2b"][:, h:h + 1])
            if h % 2 == 0:
                nc.scalar.copy(out=q2T[0:DH, h, :], in_=pq[0:DH, :])
            else:
                nc.vector.tensor_copy(q2T[0:DH, h, :], pq[0:DH, :])
        wq2p.release()
        x2p.release()

        outTn2p = tc.alloc_tile_pool(name="outTn2p", bufs=1)
        outTn2 = outTn2p.tile([P, H, S], bf16, name="outTn2")
        exp2p = tc.alloc_tile_pool(name="exp2p", bufs=3)
        dstk2p = tc.alloc_tile_pool(name="dstk2p", bufs=1)
        dstack2 = dstk2p.tile([8, 1024], bf16, name="dstack2")
        rstack2 = dstk2p.tile([8, 1024], f32r, name="rstack2")
        for h in range(H):
            ps2 = pjp2.tile([P, 1024], f32, name="ps2", tag="pj2")
            for c in range(2):
                nc.tensor.matmul(ps2[0:ENC, c * 512:(c + 1) * 512],
                                 k2T[0:DH, h, :],
                                 q2T[0:DH, h, c * 512:(c + 1) * 512],
                                 start=True, stop=True)
            ex2 = exp2p.tile([P, 1024], f32r, name="ex2", tag="exp2")
            nc.scalar.activation(ex2[0:ENC, :], ps2[0:ENC, :], AF.Exp,
                                 scale=SCALE)
            pav2 = pav2p.tile([P, 1024], f32, name="pav2", tag="pav2")
            for c in range(2):
                nc.tensor.matmul(pav2[0:DH + 1, c * 512:(c + 1) * 512],
                                 v2p[:, h * (DH + 1):(h + 1) * (DH + 1)],
                                 ex2[0:ENC, c * 512:(c + 1) * 512],
                                 start=True, stop=True)
            nc.vector.tensor_copy(outTn2[0:DH + 1, h, :], pav2[0:DH + 1, :])
            nc.gpsimd.dma_start(out=dstack2[h:h + 1, :],
                                in_=outTn2[DH:DH + 1, h, :])
        with nc.allow_low_precision(reason="f32r softmax denom recip"):
            nc.vector.reciprocal(rstack2[:], dstack2[:])
        for h in range(H):
            pb2 = pav2p.tile([P, 1024], f32, name="pb2", tag="pav2")
            for c in range(2):
                nc.tensor.matmul(pb2[0:DH, c * 512:(c + 1) * 512],
                                 ones_sel[0:8, h, :],
                                 rstack2[:, c * 512:(c + 1) * 512],
                                 start=True, stop=True)
            nc.vector.tensor_mul(outTn2[0:DH, h, :], outTn2[0:DH, h, :],
                                 pb2[0:DH, :])
        pav2p.release()
        pjp2.release()
        ptp2.release()
        dstk2p.release()
        exp2p.release()

        # o2 projection + residual -> h3
        o2pp = tc.alloc_tile_pool(name="o2pp", bufs=1)
        o2pt = o2pp.tile([P, H, D], bf16, name="o2pt")
        for h in range(H):
            nc.sync.dma_start(out=o2pt[0:DH, h, :], in_=o2p_d[h])
        pop2 = tc.alloc_tile_pool(name="pop2", bufs=2, space=PSUM)
        for t in range(NQT):
            po0 = pop2.tile([P, 512], f32, name="po20", tag="po2")
            po1 = pop2.tile([P, 512], f32, name="po21", tag="po2")
            for h in range(H):
                nc.tensor.matmul(po0[:, 0:320],
                                 outTn2[0:DH, h, t * P:(t + 1) * P],
                                 o2pt[0:DH, h, 0:320],
                                 start=(h == 0), stop=(h == H - 1))
                nc.tensor.matmul(po1[:, 0:320],
                                 outTn2[0:DH, h, t * P:(t + 1) * P],
                                 o2pt[0:DH, h, 320:640],
                                 start=(h == 0), stop=(h == H - 1))
            nc.vector.tensor_add(h3[:, t, 0:320], po0[:, 0:320],
                                 h2[:, t, 0:320])
            nc.vector.tensor_add(h3[:, t, 320:640], po1[:, 0:320],
                                 h2[:, t, 320:640])
            if has_ob2:
                nc.vector.tensor_add(h3[:, t, :], h3[:, t, :],
                                     bias_tiles["ob2"][:])
        pop2.release()
        o2pp.release()
        outTn2p.release()
        q2Tp.release()

        # ================= FFN (geglu, fp8 DoubleRow) =================
        # x3T8 holds LN3(h3)/8 in fp8 pairs: [:, p, i, :] = kt (2p+i);
        # pair slot (2, 1) is the zero pad for kt=5.
        hgTp = tc.alloc_tile_pool(name="hgTp", bufs=1)
        hgT8 = hgTp.tile([P, NP2, 2, S], fp8, name="hgT8")
        x3p = tc.alloc_tile_pool(name="x3p", bufs=1)
        x3T8 = x3p.tile([P, NP1, 2, S], fp8, name="x3T8")
        nc.vector.memset(x3T8[:, NP1 - 1, 1, :], 0.0)
        ptp3 = tc.alloc_tile_pool(name="ptp3", bufs=2, space=PSUM)
        for t in range(NQT):
            ln_block(h3[:, t, :],
                     lambda kt, t=t: x3T8[:, kt // 2, kt % 2,
                                          t * P:(t + 1) * P],
                     ptp3, rstd_mult=0.125)

        w1pp = tc.alloc_tile_pool(name="w1pp", bufs=8)
        ggp = tc.alloc_tile_pool(name="ggp", bufs=3)
        pw1 = tc.alloc_tile_pool(name="pw1", bufs=4, space=PSUM)
        for mp in range(NM1 // 2):
            wh8 = w1pp.tile([P, NP1, 2, P], fp8, name="wh8", tag="w1")
            nc.sync.dma_start(out=wh8[:], in_=w1p_d[mp])
            wg8 = w1pp.tile([P, NP1, 2, P], fp8, name="wg8", tag="w1")
            nc.sync.dma_start(out=wg8[:], in_=w1p_d[mp + NM1 // 2])
            for qc in range(2):
                ph = pw1.tile([P, 512], f32, name="ph", tag="pw1")
                pg = pw1.tile([P, 512], f32, name="pg", tag="pw1")
                for p in range(NP1):
                    nc.tensor.matmul(
                        ph[:], wh8[:, p, :, :],
                        x3T8[:, p, :, qc * 512:(qc + 1) * 512],
                        start=(p == 0), stop=(p == NP1 - 1),
                        perf_mode=DR)
                for p in range(NP1):
                    nc.tensor.matmul(
                        pg[:], wg8[:, p, :, :],
                        x3T8[:, p, :, qc * 512:(qc + 1) * 512],
                        start=(p == 0), stop=(p == NP1 - 1),
                        perf_mode=DR)
                gg = ggp.tile([P, 512], f32, name="gg", tag="gg")
                if has_b1:
                    nc.scalar.activation(
                        gg[:], pg[:], AF.Gelu_apprx_tanh,
                        bias=bias_tiles["b1p"][:, mp + NM1 // 2:
                                               mp + NM1 // 2 + 1])
                    hb = ggp.tile([P, 512], f32, name="hb", tag="gg")
                    nc.vector.tensor_scalar(
                        out=hb[:], in0=ph[:],
                        scalar1=bias_tiles["b1p"][:, mp:mp + 1],
                        scalar2=4.0, op0=Alu.add, op1=Alu.mult)
                    nc.vector.tensor_mul(
                        hgT8[:, mp // 2, mp % 2, qc * 512:(qc + 1) * 512],
                        hb[:], gg[:])
                else:
                    nc.scalar.activation(gg[:], pg[:], AF.Gelu_apprx_tanh)
                    nc.vector.scalar_tensor_tensor(
                        out=hgT8[:, mp // 2, mp % 2,
                                 qc * 512:(qc + 1) * 512],
                        in0=ph[:], scalar=4.0, in1=gg[:],
                        op0=Alu.mult, op1=Alu.mult)
        pw1.release()
        ggp.release()
        w1pp.release()
        x3p.release()
        ptp3.release()

        pw2 = tc.alloc_tile_pool(name="pw2", bufs=4, space=PSUM)
        for th in range(4):
            pf = []
            for tt in range(2):
                pf.append((pw2.tile([P, 320], f32, name=f"pf{tt}a",
                                    tag="pw2"),
                           pw2.tile([P, 320], f32, name=f"pf{tt}b",
                                    tag="pw2")))
            for pr in range(NP2):
                for tt in range(2):
                    t = th * 2 + tt
                    nc.tensor.matmul(pf[tt][0][:],
                                     hgT8[:, pr, :, t * P:(t + 1) * P],
                                     w2sb[:, pr, :, 0:320],
                                     start=(pr == 0), stop=(pr == NP2 - 1),
                                     perf_mode=DR)
                    nc.tensor.matmul(pf[tt][1][:],
                                     hgT8[:, pr, :, t * P:(t + 1) * P],
                                     w2sb[:, pr, :, 320:640],
                                     start=(pr == 0), stop=(pr == NP2 - 1),
                                     perf_mode=DR)
            for tt in range(2):
                t = th * 2 + tt
                ot = io.tile([P, D], f32, name="ot", tag="io")
                nc.vector.scalar_tensor_tensor(
                    out=ot[:, 0:320], in0=pf[tt][0][:], scalar=1.0 / 64,
                    in1=h3[:, t, 0:320], op0=Alu.mult, op1=Alu.add)
                nc.vector.scalar_tensor_tensor(
                    out=ot[:, 320:640], in0=pf[tt][1][:], scalar=1.0 / 64,
                    in1=h3[:, t, 320:640], op0=Alu.mult, op1=Alu.add)
                if has_fb2:
                    nc.vector.tensor_add(ot[:], ot[:], bias_tiles["fb2"][:])
                nc.sync.dma_start(out=out_d[t * P:(t + 1) * P, :], in_=ot[:])
        pw2.release()
        hgTp.release()

        h3p.release()
        w2sbp.release()
        v2pp.release()
        k2Tp.release()
        h2p.release()
        xop.release()
        io.release()
        stats.release()
        const.release()

    nc.compile()
    return nc


def _prep_inputs(inputs):
    import ml_dtypes

    f32 = np.float32
    bf = ml_dtypes.bfloat16
    f8 = ml_dtypes.float8_e4m3
    g = {k: np.asarray(v) for k, v in inputs.items()}
    hs = np.ascontiguousarray(g["hidden_states"], f32)
    enc = np.ascontiguousarray(g["encoder_hidden_states"], f32)
    f = int(g["video_length"])
    assert hs.shape == (F, S, D) and enc.shape == (F, ENC, CROSS) and f == F

    ln1w, ln1b = g["ln1_w"].astype(f32), g["ln1_b"].astype(f32)
    ln2w, ln2b = g["ln2_w"].astype(f32), g["ln2_b"].astype(f32)
    ln3w, ln3b = g["ln3_w"].astype(f32), g["ln3_b"].astype(f32)
    q1, k1, v1 = (g[n].astype(f32) for n in ("q1", "k1", "v1"))
    o1w, o1b = g["o1_w"].astype(f32), g["o1_b"].astype(f32)
    q2, k2, v2 = (g[n].astype(f32) for n in ("q2", "k2", "v2"))
    o2w, o2b = g["o2_w"].astype(f32), g["o2_b"].astype(f32)
    w1, b1 = g["ff_w1"].astype(f32), g["ff_b1"].astype(f32)
    w2, b2 = g["ff_w2"].astype(f32), g["ff_b2"].astype(f32)

    # ff_w1: fold LN3 scale, multiply by 8 (x3 is pre-scaled by 1/8), pad
    # the 640-dim contraction to 768 and interleave into DoubleRow pairs.
    w1f = (w1 * ln3w[:, None]) * 8.0
    w1pad = np.zeros((768, 2 * FFI), f32)
    w1pad[:D] = w1f
    w1p8 = np.ascontiguousarray(
        w1pad.reshape(NP1, 2, P, NM1, P).transpose(3, 2, 0, 1, 4)).astype(f8)
    # ff_w2: *16 (geglu product carries *4; total 64 undone on output).
    # Layout [P, NP2, 2, D]: partition-major so the whole tensor loads in
    # one contiguous DMA at program start.
    w2p8 = np.ascontiguousarray(
        (w2 * 16.0).reshape(NP2, 2, P, D).transpose(2, 0, 1, 3)).astype(f8)

    shared = {
        "wq1": np.ascontiguousarray(q1 * ln1w[:, None]).astype(bf),
        "wk1": np.ascontiguousarray(k1 * ln1w[:, None]).astype(bf),
        "wv1": np.ascontiguousarray(v1 * ln1w[:, None]).astype(bf),
        "o1p": np.ascontiguousarray(o1w.reshape(H, DH, D)).astype(bf),
        "wq2": np.ascontiguousarray(q2 * ln2w[:, None]).astype(bf),
        "wk2": np.ascontiguousarray(k2),
        "wv2": np.ascontiguousarray(v2),
        "o2p": np.ascontiguousarray(o2w.reshape(H, DH, D)).astype(bf),
        "w1p8": w1p8,
        "w2p8": w2p8,
    }

    qb1 = ln1b @ q1
    kb1 = ln1b @ k1
    vb1 = ln1b @ v1
    q2b = ln2b @ q2
    b1f = b1 + ln3b @ w1
    flags = (
        bool(np.any(qb1)), bool(np.any(kb1)), bool(np.any(vb1)),
        bool(np.any(o1b)), bool(np.any(q2b)), bool(np.any(o2b)),
        bool(np.any(b1f)), bool(np.any(b2)),
    )
    (has_qb1, has_kb1, has_vb1, has_ob1, has_q2b, has_ob2, has_b1,
     has_fb2) = flags
    if has_b1:
        shared["b1p"] = np.ascontiguousarray(b1f.reshape(NM1, P).T)
    if has_qb1:
        shared["qb1"] = np.ascontiguousarray(qb1.reshape(H, DH).T)
    if has_kb1:
        shared["kb1"] = np.ascontiguousarray(kb1.reshape(H, DH).T)
    if has_vb1:
        shared["vb1"] = np.ascontiguousarray(vb1.reshape(H, DH).T)
    if has_q2b:
        shared["q2b"] = np.ascontiguousarray(q2b.reshape(H, DH).T)
    if has_ob1:
        shared["ob1"] = np.ascontiguousarray(np.broadcast_to(o1b, (P, D)))
    if has_ob2:
        shared["ob2"] = np.ascontiguousarray(np.broadcast_to(o2b, (P, D)))
    if has_fb2:
        shared["fb2"] = np.ascontiguousarray(np.broadcast_to(b2, (P, D)))

    former = [0] + list(range(F - 1))
    in_maps = []
    for i in range(F):
        m = dict(shared)
        m["hs3"] = np.ascontiguousarray(
            np.concatenate([hs[i], hs[0], hs[former[i]]], axis=0))
        m["enc"] = np.ascontiguousarray(enc[i])
        in_maps.append(m)
    return flags, in_maps


def get_program(flags):
    if flags not in _PROGRAM_CACHE:
        _PROGRAM_CACHE[flags] = _build_program(flags)
    return _PROGRAM_CACHE[flags]


def run(inputs, trace=False):
    from concourse.bass_utils import run_bass_kernel_spmd

    flags, in_maps = _prep_inputs(inputs)
    nc = get_program(flags)
    res = run_bass_kernel_spmd(nc, in_maps, core_ids=list(range(F)),
                               trace=trace)
    out = np.stack([r["out"] for r in res.results], axis=0)
    return out.astype(np.float32), res


def kernel(**inputs):
    out, _ = run(inputs, trace=False)
    return out


# revision 62
# speedup vs baseline: 1.0200x; 1.0200x over previous
"""Trainium2 Bass kernel for a sparse-causal-attention BasicTransformerBlock.

Sharding: pure data-parallel over the 8 video frames (batch=1, f=8) -- one
frame per NeuronCore, zero collectives.  Each core receives its own frame
plus frame 0 and the previous frame (the sparse-causal KV sources) and
recomputes LN1 + K/V projections for those locally.

v2 design (vs the first working version):
  - every matmul operand is bf16 (fp8 for the FFN) -> half-cost LDWEIGHTS,
    half weight DMA traffic; PSUM accumulation stays fp32.
  - softmax normalization: denominator rows are stacked into a [8, q] tile,
    one batched DVE reciprocal per q-half, broadcast across partitions with
    a K=8 one-hot PE matmul (no DRAM round-trips, no per-head reciprocal).
  - encoder-side attn2 work (enc transpose, K2/V2 projections) hoisted to
    the program start to fill the PE while hs3 streams in.
  - FFN runs fp8e4 DoubleRow matmuls (2 contraction rows/cycle).  To dodge
    fp8 denormals the LN3 output is pre-scaled by 1/8 and w1 by 8 (exact
    cancellation); the geglu product is scaled by 4 and w2 by 16, undone
    with a single *1/64 in the final residual add.
"""

import numpy as np

P = 128
S = 1024          # tokens per frame
D = 640
H = 8
DH = 80
KV = 2 * S        # sparse-causal kv tokens (first frame + prev frame)
ENC = 77
CROSS = 768
FFI = 2560
NQT = S // P      # 8 token tiles
NKT = KV // P     # 16 kv token tiles
ND = D // P       # 5 dim tiles
NE = CROSS // P   # 6 encoder-dim tiles
NM1 = 2 * FFI // P  # 40 ff_w1 out tiles
NP1 = 3             # ff_w1 contraction pairs (640 -> padded 768 = 3*256)
NP2 = FFI // 256    # 10 ff_w2 contraction pairs
SCALE = DH ** -0.5
EPS = 1e-5
F = 8

_PROGRAM_CACHE = {}


def _build_program(flags):
    import concourse.bass as bass
    import concourse.tile as tile
    from concourse import bacc, mybir
    from concourse.masks import make_identity

    f32 = mybir.dt.float32
    f32r = mybir.dt.float32r
    bf16 = mybir.dt.bfloat16
    fp8 = mybir.dt.float8e4
    AF = mybir.ActivationFunctionType
    Alu = mybir.AluOpType
    DR = mybir.MatmulPerfMode.DoubleRow
    PSUM = bass.MemorySpace.PSUM

    (has_qb1, has_kb1, has_vb1, has_ob1, has_q2b, has_ob2, has_b1,
     has_fb2) = flags

    nc = bacc.Bacc(None, target_bir_lowering=False)

    hs3_d = nc.dram_tensor("hs3", [3 * S, D], f32, kind="ExternalInput")[:]
    enc_d = nc.dram_tensor("enc", [ENC, CROSS], f32, kind="ExternalInput")[:]
    wq1_d = nc.dram_tensor("wq1", [D, D], bf16, kind="ExternalInput")[:]
    wk1_d = nc.dram_tensor("wk1", [D, D], bf16, kind="ExternalInput")[:]
    wv1_d = nc.dram_tensor("wv1", [D, D], bf16, kind="ExternalInput")[:]
    o1p_d = nc.dram_tensor("o1p", [H, DH, D], bf16, kind="ExternalInput")[:]
    wq2_d = nc.dram_tensor("wq2", [D, D], bf16, kind="ExternalInput")[:]
    wk2_d = nc.dram_tensor("wk2", [CROSS, D], f32r, kind="ExternalInput")[:]
    wv2_d = nc.dram_tensor("wv2", [CROSS, D], f32r, kind="ExternalInput")[:]
    o2p_d = nc.dram_tensor("o2p", [H, DH, D], bf16, kind="ExternalInput")[:]
    w1p_d = nc.dram_tensor("w1p8", [NM1, P, NP1, 2, P], fp8,
                           kind="ExternalInput")[:]
    w2p_d = nc.dram_tensor("w2p8", [P, NP2, 2, D], fp8,
                           kind="ExternalInput")[:]
    out_d = nc.dram_tensor("out", [S, D], f32, kind="ExternalOutput")[:]

    b1p_d = qb1_d = kb1_d = vb1_d = q2b_d = None
    ob1_d = ob2_d = fb2_d = None
    if has_b1:
        b1p_d = nc.dram_tensor("b1p", [P, NM1], f32, kind="ExternalInput")[:]
    if has_qb1:
        qb1_d = nc.dram_tensor("qb1", [DH, H], f32, kind="ExternalInput")[:]
    if has_kb1:
        kb1_d = nc.dram_tensor("kb1", [DH, H], f32, kind="ExternalInput")[:]
    if has_vb1:
        vb1_d = nc.dram_tensor("vb1", [DH, H], f32, kind="ExternalInput")[:]
    if has_q2b:
        q2b_d = nc.dram_tensor("q2b", [DH, H], f32, kind="ExternalInput")[:]
    if has_ob1:
        ob1_d = nc.dram_tensor("ob1", [P, D], f32, kind="ExternalInput")[:]
    if has_ob2:
        ob2_d = nc.dram_tensor("ob2", [P, D], f32, kind="ExternalInput")[:]
    if has_fb2:
        fb2_d = nc.dram_tensor("fb2", [P, D], f32, kind="ExternalInput")[:]

    with tile.TileContext(nc) as tc:
        # ---------- whole-kernel constants ----------
        const = tc.alloc_tile_pool(name="const", bufs=1)
        idb = const.tile([P, P], bf16, name="idb")
        make_identity(nc, idb)
        idf = const.tile([P, P], f32, name="idf")
        make_identity(nc, idf)
        epst = const.tile([P, 1], f32, name="epst")
        nc.vector.memset(epst[:], EPS)
        ones_f32 = const.tile([P, DH], f32, name="ones_f32")
        nc.vector.memset(ones_f32[:], 1.0)
        onest = const.tile([1, DH], f32r, name="onest")
        nc.vector.tensor_copy(onest[:], ones_f32[0:1, :])
        # one-hot rows for the denominator broadcast: sel[:, h, :] has row h
        # all-ones (out = recip[h] broadcast over DH partitions).  Built with
        # affine_select (block identity): keep 0 where (x - h) != 0 else 1.
        ones_sel = const.tile([8, H, DH], f32r, name="ones_sel")
        ones_scr = const.tile([8, H, DH], f32, name="ones_scr")
        nc.gpsimd.memset(ones_scr[:], 0.0)
        nc.gpsimd.affine_select(
            out=ones_scr[:], in_=ones_scr[:],
            compare_op=Alu.not_equal, fill=1.0, base=0,
            pattern=[[-1, H], [0, DH]], channel_multiplier=1)
        nc.vector.tensor_copy(ones_sel[:], ones_scr[:])
        bias_tiles = {}
        if has_b1:
            t = const.tile([P, NM1], f32, name="b1pt")
            nc.sync.dma_start(out=t[:], in_=b1p_d)
            bias_tiles["b1p"] = t
        for nm, dref in (("qb1", qb1_d), ("kb1", kb1_d), ("vb1", vb1_d),
                         ("q2b", q2b_d)):
            if dref is not None:
                t = const.tile([DH, H], f32, name=nm + "t")
                nc.sync.dma_start(out=t[:], in_=dref)
                bias_tiles[nm] = t
        for nm, dref in (("ob1", ob1_d), ("ob2", ob2_d), ("fb2", fb2_d)):
            if dref is not None:
                t = const.tile([P, D], f32, name=nm + "t")
                nc.sync.dma_start(out=t[:], in_=dref)
                bias_tiles[nm] = t

        stats = tc.alloc_tile_pool(name="stats", bufs=6)
        io = tc.alloc_tile_pool(name="io", bufs=3)
        xop = tc.alloc_tile_pool(name="xop", bufs=3)
        h2p = tc.alloc_tile_pool(name="h2p", bufs=1)
        h2 = h2p.tile([P, NQT, D], f32, name="h2")

        def ln_block(xin, xT_dst, pt_pool, rstd_mult=None):
            """LayerNorm (scale/bias folded into consuming weights) and
            per-128-block PE transpose of the bf16 output.

            xin [128, 640] f32 sbuf; xT_dst(kt) -> [128,128] bf16 dst AP."""
            st = stats.tile([P, 2, 6], f32, name="st", tag="st")
            nc.vector.bn_stats(st[:, 0, :], xin[:, 0:320])
            nc.vector.bn_stats(st[:, 1, :], xin[:, 320:640])
            mv = stats.tile([P, 2], f32, name="mv", tag="mv")
            nc.vector.bn_aggr(mv[:], st[:])
            rstd = stats.tile([P, 1], f32, name="rstd", tag="rstd")
            nc.scalar.activation(rstd[:], mv[:, 1:2], AF.Sqrt, bias=epst[:])
            nc.vector.reciprocal(rstd[:], rstd[:])
            if rstd_mult is not None:
                nc.vector.tensor_scalar(
                    out=rstd[:], in0=rstd[:], scalar1=rstd_mult,
                    scalar2=None, op0=Alu.mult, op1=Alu.bypass)
            mb = stats.tile([P, 1], f32, name="mb", tag="mb")
            nc.vector.tensor_scalar(
                out=mb[:], in0=mv[:, 0:1], scalar1=rstd[:], scalar2=-1.0,
                op0=Alu.mult, op1=Alu.mult)
            xo = xop.tile([P, D], bf16, name="xo", tag="xo")
            nc.scalar.activation(xo[:], xin, AF.Identity,
                                 scale=rstd[:], bias=mb[:])
            for kt in range(ND):
                ptile = pt_pool.tile([P, P], bf16, name="ptile", tag="pt")
                nc.tensor.transpose(ptile[:], xo[:, kt * P:(kt + 1) * P],
                                    idb[:])
                if kt % 2 == 0:
                    nc.scalar.copy(out=xT_dst(kt), in_=ptile[:])
                else:
                    nc.vector.tensor_copy(xT_dst(kt), ptile[:])

        # ================= encoder-side attn2 prep (fills PE early) ======
        k2Tp = tc.alloc_tile_pool(name="k2Tp", bufs=1)
        k2T = k2Tp.tile([P, H, ENC], bf16, name="k2T")
        v2pp = tc.alloc_tile_pool(name="v2pp", bufs=1)
        v2p = v2pp.tile([ENC, H * (DH + 1)], f32r, name="v2p")
        encTp = tc.alloc_tile_pool(name="encTp", bufs=1)
        # fp32r matmuls need an even moving-dim size; pad 77 -> 78 zeros
        encT = encTp.tile([P, NE, ENC + 1], f32r, name="encT")
        zcol = encTp.tile([P, NE], f32, name="zcol")
        nc.vector.memset(zcol[:], 0.0)
        nc.vector.tensor_copy(
            encT[:, :, ENC:ENC + 1],
            zcol[:].rearrange("p (a b) -> p a b", b=1))
        encp = tc.alloc_tile_pool(name="encp", bufs=1)
        enc_sb = encp.tile([ENC, CROSS], f32, name="enc_sb")
        nc.sync.dma_start(out=enc_sb[:], in_=enc_d)
        ptpe = tc.alloc_tile_pool(name="ptpe", bufs=2, space=PSUM)
        for kt in range(NE):
            ptile = ptpe.tile([P, P], f32, name="ptileE", tag="pt")
            nc.tensor.transpose(ptile[0:P, 0:ENC],
                                enc_sb[:, kt * P:(kt + 1) * P],
                                idf[0:ENC, 0:ENC])
            nc.vector.tensor_copy(encT[:, kt, 0:ENC], ptile[0:P, 0:ENC])

        wk2p = tc.alloc_tile_pool(name="wk2p", bufs=6)
        pje = tc.alloc_tile_pool(name="pje", bufs=2, space=PSUM)
        wk2 = []
        for kt in range(NE):
            w = wk2p.tile([P, D], f32r, name=f"wk2{kt}", tag="w")
            nc.sync.dma_start(out=w[:], in_=wk2_d[kt * P:(kt + 1) * P, :])
            wk2.append(w)
        for h in range(H):
            pk2 = pje.tile([P, P], f32, name="pk2", tag="pj")
            for kt in range(NE):
                nc.tensor.matmul(pk2[0:DH, 0:ENC + 1],
                                 wk2[kt][:, h * DH:(h + 1) * DH],
                                 encT[:, kt, :],
                                 start=(kt == 0), stop=(kt == NE - 1))
            nc.scalar.copy(out=k2T[0:DH, h, :], in_=pk2[0:DH, 0:ENC])
        wv2 = []
        for kt in range(NE):
            w = wk2p.tile([P, D], f32r, name=f"wv2{kt}", tag="w")
            nc.sync.dma_start(out=w[:], in_=wv2_d[kt * P:(kt + 1) * P, :])
            wv2.append(w)
        pv20 = pje.tile([P, 320], f32, name="pv20", tag="pj")
        pv21 = pje.tile([P, 320], f32, name="pv21", tag="pj")
        for kt in range(NE):
            nc.tensor.matmul(pv20[0:ENC, :], encT[:, kt, 0:ENC],
                             wv2[kt][:, 0:320],
                             start=(kt == 0), stop=(kt == NE - 1))
            nc.tensor.matmul(pv21[0:ENC, :], encT[:, kt, 0:ENC],
                             wv2[kt][:, 320:640],
                             start=(kt == 0), stop=(kt == NE - 1))
        v2sl = v2p[:].rearrange("p (a b) -> p a b", b=DH + 1)
        nc.vector.tensor_copy(v2sl[:, 0:4, 0:DH],
                              pv20[0:ENC, :].rearrange("p (a b) -> p a b",
                                                       b=DH))
        nc.vector.tensor_copy(v2sl[:, 4:8, 0:DH],
                              pv21[0:ENC, :].rearrange("p (a b) -> p a b",
                                                       b=DH))
        nc.vector.tensor_copy(
            v2sl[:, :, DH:DH + 1],
            ones_f32[0:ENC, 0:H].rearrange("p (a b) -> p a b", b=1))
        wk2p.release()
        pje.release()
        ptpe.release()
        encp.release()
        encTp.release()

        # ================= LN1 + QKV projections =================
        w2sbp = tc.alloc_tile_pool(name="w2sbp", bufs=1)
        w2sb = w2sbp.tile([P, NP2, 2, D], fp8, name="w2sb")
        nc.sync.dma_start(out=w2sb[:], in_=w2p_d)
        kTp = tc.alloc_tile_pool(name="kTp", bufs=1)
        kT = kTp.tile([P, H, KV], bf16, name="kT")
        vpp = tc.alloc_tile_pool(name="vpp", bufs=1)
        vp = vpp.tile([P, NKT, H * (DH + 1)], bf16, name="vp")
        qTp = tc.alloc_tile_pool(name="qTp", bufs=1)
        qT = qTp.tile([P, H, S], bf16, name="qT")

        xowp = tc.alloc_tile_pool(name="xowp", bufs=1)
        xowT = xowp.tile([P, ND, S], bf16, name="xowT")
        xkvp = tc.alloc_tile_pool(name="xkvp", bufs=1)
        xkvT = xkvp.tile([P, ND, KV], bf16, name="xkvT")
        ptp = tc.alloc_tile_pool(name="ptp", bufs=2, space=PSUM)
        pjp = tc.alloc_tile_pool(name="pjp", bufs=2, space=PSUM)
        pvp = tc.alloc_tile_pool(name="pvp", bufs=2, space=PSUM)
        wkvp = tc.alloc_tile_pool(name="wkvp", bufs=6)

        # own frame (rows 0:1024) -> LN -> xowT
        for t in range(NQT):
            xt = io.tile([P, D], f32, name="xt", tag="io")
            nc.sync.dma_start(out=xt[:], in_=hs3_d[t * P:(t + 1) * P, :])
            ln_block(xt[:],
                     lambda kt, t=t: xowT[:, kt, t * P:(t + 1) * P], ptp)
        # kv frames (rows 1024:3072) -> LN -> xkvT
        for t in range(NKT):
            xt = io.tile([P, D], f32, name="xt", tag="io")
            nc.sync.dma_start(out=xt[:],
                              in_=hs3_d[(NQT + t) * P:(NQT + t + 1) * P, :])
            ln_block(xt[:],
                     lambda kt, t=t: xkvT[:, kt, t * P:(t + 1) * P], ptp)

        # Q projection (per head; moving = xowT chunks)
        wq = []
        for kt in range(ND):
            w = wkvp.tile([P, D], bf16, name=f"wq{kt}", tag="w")
            nc.sync.dma_start(out=w[:], in_=wq1_d[kt * P:(kt + 1) * P, :])
            wq.append(w)
        for h in range(H):
            pq = pjp.tile([P, 1024], f32, name="pq", tag="pj")
            for c in range(2):
                for kt in range(ND):
                    nc.tensor.matmul(
                        pq[0:DH, c * 512:(c + 1) * 512],
                        wq[kt][:, h * DH:(h + 1) * DH],
                        xowT[:, kt, c * 512:(c + 1) * 512],
                        start=(kt == 0), stop=(kt == ND - 1))
            if has_qb1:
                nc.vector.tensor_scalar_add(
                    pq[0:DH, :], pq[0:DH, :], bias_tiles["qb1"][:, h:h + 1])
            if h % 2 == 0:
                nc.scalar.copy(out=qT[0:DH, h, :], in_=pq[0:DH, :])
            else:
                nc.vector.tensor_copy(qT[0:DH, h, :], pq[0:DH, :])

        # V projection (stationary = xkvT tiles, moving = wv)
        wv = []
        for kt in range(ND):
            w = wkvp.tile([P, D], bf16, name=f"wv{kt}", tag="w")
            nc.sync.dma_start(out=w[:], in_=wv1_d[kt * P:(kt + 1) * P, :])
            wv.append(w)
        for m in range(NKT):
            pv0 = pvp.tile([P, 320], f32, name="pv0", tag="pv")
            pv1 = pvp.tile([P, 320], f32, name="pv1", tag="pv")
            for kt in range(ND):
                nc.tensor.matmul(pv0[:], xkvT[:, kt, m * P:(m + 1) * P],
                                 wv[kt][:, 0:320],
                                 start=(kt == 0), stop=(kt == ND - 1))
                nc.tensor.matmul(pv1[:], xkvT[:, kt, m * P:(m + 1) * P],
                                 wv[kt][:, 320:640],
                                 start=(kt == 0), stop=(kt == ND - 1))
            vsl = vp[:, m, :].rearrange("p (a b) -> p a b", b=DH + 1)
            nc.vector.tensor_copy(
                vsl[:, 0:4, 0:DH], pv0[:].rearrange("p (a b) -> p a b", b=DH))
            nc.vector.tensor_copy(
                vsl[:, 4:8, 0:DH], pv1[:].rearrange("p (a b) -> p a b", b=DH))
            nc.vector.memset(vsl[:, :, DH:DH + 1], 1.0)

        wk = []
        for kt in range(ND):
            w = wkvp.tile([P, D], bf16, name=f"wk{kt}", tag="w")
            nc.sync.dma_start(out=w[:], in_=wk1_d[kt * P:(kt + 1) * P, :])
            wk.append(w)
        pvp.release()
        pjp.release()
        ptp.release()

        # ======== attn1: K-proj fused into the per-head attention loop ====
        # Head h's attention starts as soon as its own K tiles are done,
        # instead of waiting for all 8 heads' projections.
        outTnp = tc.alloc_tile_pool(name="outTnp", bufs=1)
        outTn = outTnp.tile([P, H, S], bf16, name="outTn")
        o1pp = tc.alloc_tile_pool(name="o1pp", bufs=1)
        o1pt = o1pp.tile([P, H, D], bf16, name="o1pt")
        for h in range(H):
            nc.sync.dma_start(out=o1pt[0:DH, h, :], in_=o1p_d[h])
        expp = tc.alloc_tile_pool(name="expp", bufs=4)
        dstkp = tc.alloc_tile_pool(name="dstkp", bufs=4)
        psp = tc.alloc_tile_pool(name="psp", bufs=2, space=PSUM)
        pavp = tc.alloc_tile_pool(name="pavp", bufs=2, space=PSUM)
        popb = tc.alloc_tile_pool(name="popb", bufs=2, space=PSUM)

        dstacks = [dstkp.tile([8, 512], bf16, name=f"dstack{qc}", tag="dst")
                   for qc in range(2)]
        rstacks = [dstkp.tile([8, 512], f32r, name=f"rstack{qc}", tag="rst")
                   for qc in range(2)]
        for h in range(H):
            # K projection for this head.  PSUM comes from the popb pool
            # (idle until the normalization tail) so K-proj copies never
            # starve the scores pipeline of ps slots.
            for c4 in range(4):
                pk = popb.tile([P, 512], f32, name="pk", tag="popb")
                for kt in range(ND):
                    nc.tensor.matmul(
                        pk[0:DH, :],
                        wk[kt][:, h * DH:(h + 1) * DH],
                        xkvT[:, kt, c4 * 512:(c4 + 1) * 512],
                        start=(kt == 0), stop=(kt == ND - 1))
                if has_kb1:
                    nc.vector.tensor_scalar_add(
                        pk[0:DH, :], pk[0:DH, :],
                        bias_tiles["kb1"][:, h:h + 1])
                nc.vector.tensor_copy(
                    kT[0:DH, h, c4 * 512:(c4 + 1) * 512], pk[0:DH, :])
            for qc in range(2):
                pav = pavp.tile([P, 512], f32, name="pav", tag="pav")
                for kp in range(NKT // 2):
                    ps = psp.tile([P, 1024], f32, name="ps", tag="ps")
                    for half in range(2):
                        kvt = 2 * kp + half
                        nc.tensor.matmul(
                            ps[:, half * 512:(half + 1) * 512],
                            kT[0:DH, h, kvt * P:(kvt + 1) * P],
                            qT[0:DH, h, qc * 512:(qc + 1) * 512],
                            start=True, stop=True)
                    ex = expp.tile([P, 1024], bf16, name="ex", tag="exp")
                    nc.scalar.activation(ex[:], ps[:], AF.Exp, scale=SCALE)
                    for half in range(2):
                        kvt = 2 * kp + half
                        nc.tensor.matmul(
                            pav[0:DH + 1, :],
                            vp[:, kvt, h * (DH + 1):(h + 1) * (DH + 1)],
                            ex[:, half * 512:(half + 1) * 512],
                            start=(kvt == 0), stop=(kvt == NKT - 1))
                qs = slice(qc * 512, (qc + 1) * 512)
                nc.vector.tensor_copy(outTn[0:DH + 1, h, qs],
                                      pav[0:DH + 1, :])
                nc.gpsimd.dma_start(out=dstacks[qc][h:h + 1, :],
                                    in_=outTn[DH:DH + 1, h, qs])
        for qc in range(2):
            with nc.allow_low_precision(reason="f32r softmax denom recip"):
                nc.vector.reciprocal(rstacks[qc][:], dstacks[qc][:])
            for h in range(H):
                qs = slice(qc * 512, (qc + 1) * 512)
                pb = popb.tile([P, 512], f32, name="pb", tag="popb")
                nc.tensor.matmul(pb[0:DH, :], ones_sel[0:8, h, :],
                                 rstacks[qc][:], start=True, stop=True)
                nc.vector.tensor_mul(outTn[0:DH, h, qs],
                                     outTn[0:DH, h, qs], pb[0:DH, :])
                if has_vb1:
                    nc.vector.tensor_scalar_add(
                        outTn[0:DH, h, qc * 512:(qc + 1) * 512],
                        outTn[0:DH, h, qc * 512:(qc + 1) * 512],
                        bias_tiles["vb1"][:, h:h + 1])
        # o1 projection + residual
        for t in range(NQT):
            hres = io.tile([P, D], f32, name="hres", tag="io")
            nc.sync.dma_start(out=hres[:], in_=hs3_d[t * P:(t + 1) * P, :])
            # o1 accumulators live in the (now idle) ps pool as bank-aligned
            # halves of one [P,1024] tile -> 2 tokens in flight keeps the PE
            # dense enough that the HAM clock gate stays at full rate.
            po = psp.tile([P, 1024], f32, name="po", tag="ps")
            for h in range(H):
                nc.tensor.matmul(po[:, 0:320],
                                 outTn[0:DH, h, t * P:(t + 1) * P],
                                 o1pt[0:DH, h, 0:320],
                                 start=(h == 0), stop=(h == H - 1))
                nc.tensor.matmul(po[:, 512:832],
                                 outTn[0:DH, h, t * P:(t + 1) * P],
                                 o1pt[0:DH, h, 320:640],
                                 start=(h == 0), stop=(h == H - 1))
            nc.vector.tensor_add(h2[:, t, 0:320], po[:, 0:320],
                                 hres[:, 0:320])
            nc.vector.tensor_add(h2[:, t, 320:640], po[:, 512:832],
                                 hres[:, 320:640])
            if has_ob1:
                nc.vector.tensor_add(h2[:, t, :], h2[:, t, :],
                                     bias_tiles["ob1"][:])
        popb.release()
        pavp.release()
        psp.release()
        dstkp.release()
        expp.release()
        o1pp.release()
        outTnp.release()
        wkvp.release()
        xkvp.release()
        xowp.release()
        qTp.release()
        vpp.release()
        kTp.release()

        # ================= attn2: cross attention =================
        h3p = tc.alloc_tile_pool(name="h3p", bufs=1)
        h3 = h3p.tile([P, NQT, D], f32, name="h3")
        q2Tp = tc.alloc_tile_pool(name="q2Tp", bufs=1)
        q2T = q2Tp.tile([P, H, S], bf16, name="q2T")
        x2p = tc.alloc_tile_pool(name="x2p", bufs=1)
        x2T = x2p.tile([P, ND, S], bf16, name="x2T")
        ptp2 = tc.alloc_tile_pool(name="ptp2", bufs=2, space=PSUM)
        pjp2 = tc.alloc_tile_pool(name="pjp2", bufs=1, space=PSUM)
        pav2p = tc.alloc_tile_pool(name="pav2p", bufs=2, space=PSUM)
        for t in range(NQT):
            ln_block(h2[:, t, :],
                     lambda kt, t=t: x2T[:, kt, t * P:(t + 1) * P], ptp2)

        wq2p = tc.alloc_tile_pool(name="wq2p", bufs=5)
        wq2 = []
        for kt in range(ND):
            w = wq2p.tile([P, D], bf16, name=f"wq2{kt}", tag="w")
            nc.sync.dma_start(out=w[:], in_=wq2_d[kt * P:(kt + 1) * P, :])
            wq2.append(w)
        for h in range(H):
            # pav2p is idle until scores start -> borrow it so q2 heads
            # double-buffer instead of serializing on the single pjp2 slot.
            pq = pav2p.tile([P, 1024], f32, name="pq2", tag="pav2")
            for c in range(2):
                for kt in range(ND):
                    nc.tensor.matmul(
                        pq[0:DH, c * 512:(c + 1) * 512],
                        wq2[kt][:, h * DH:(h + 1) * DH],
                        x2T[:, kt, c * 512:(c + 1) * 512],
                        start=(kt == 0), stop=(kt == ND - 1))
            if has_q2b:
                nc.vector.tensor_scalar_add(
                    pq[0:DH, :], pq[0:DH, :], bias_tiles["q2b"][:, h:h + 1])
            if h % 2 == 0:
                nc.scalar.copy(out=q2T[0:DH, h, :], in_=pq[0:DH, :])
            else:
                nc.vector.tensor_copy(q2T[0:DH, h, :], pq[0:DH, :])
        wq2p.release()
        x2p.release()

        outTn2p = tc.alloc_tile_pool(name="outTn2p", bufs=1)
        outTn2 = outTn2p.tile([P, H, S], bf16, name="outTn2")
        exp2p = tc.alloc_tile_pool(name="exp2p", bufs=3)
        dstk2p = tc.alloc_tile_pool(name="dstk2p", bufs=1)
        dstack2 = dstk2p.tile([8, 1024], bf16, name="dstack2")
        rstack2 = dstk2p.tile([8, 1024], f32r, name="rstack2")
        for h in range(H):
            ps2 = pjp2.tile([P, 1024], f32, name="ps2", tag="pj2")
            for c in range(2):
                nc.tensor.matmul(ps2[0:ENC, c * 512:(c + 1) * 512],
                                 k2T[0:DH, h, :],
                                 q2T[0:DH, h, c * 512:(c + 1) * 512],
                                 start=True, stop=True)
            ex2 = exp2p.tile([P, 1024], f32r, name="ex2", tag="exp2")
            nc.scalar.activation(ex2[0:ENC, :], ps2[0:ENC, :], AF.Exp,
                                 scale=SCALE)
            pav2 = pav2p.tile([P, 1024], f32, name="pav2", tag="pav2")
            for c in range(2):
                nc.tensor.matmul(pav2[0:DH + 1, c * 512:(c + 1) * 512],
                                 v2p[:, h * (DH + 1):(h + 1) * (DH + 1)],
                                 ex2[0:ENC, c * 512:(c + 1) * 512],
                                 start=True, stop=True)
            nc.vector.tensor_copy(outTn2[0:DH + 1, h, :], pav2[0:DH + 1, :])
            nc.gpsimd.dma_start(out=dstack2[h:h + 1, :],
                                in_=outTn2[DH:DH + 1, h, :])
        with nc.allow_low_precision(reason="f32r softmax denom recip"):
            nc.vector.reciprocal(rstack2[:], dstack2[:])
        for h in range(H):
            pb2 = pav2p.tile([P, 1024], f32, name="pb2", tag="pav2")
            for c in range(2):
                nc.tensor.matmul(pb2[0:DH, c * 512:(c + 1) * 512],
                                 ones_sel[0:8, h, :],
                                 rstack2[:, c * 512:(c + 1) * 512],
                                 start=True, stop=True)
            nc.vector.tensor_mul(outTn2[0:DH, h, :], outTn2[0:DH, h, :],
                                 pb2[0:DH, :])
        pav2p.release()
        pjp2.release()
        ptp2.release()
        dstk2p.release()
        exp2p.release()

        # o2 projection + residual -> h3
        o2pp = tc.alloc_tile_pool(name="o2pp", bufs=1)
        o2pt = o2pp.tile([P, H, D], bf16, name="o2pt")
        for h in range(H):
            nc.sync.dma_start(out=o2pt[0:DH, h, :], in_=o2p_d[h])
        pop2 = tc.alloc_tile_pool(name="pop2", bufs=4, space=PSUM)
        for t in range(NQT):
            po = pop2.tile([P, 1024], f32, name="po2", tag="po2")
            for h in range(H):
                nc.tensor.matmul(po[:, 0:320],
                                 outTn2[0:DH, h, t * P:(t + 1) * P],
                                 o2pt[0:DH, h, 0:320],
                                 start=(h == 0), stop=(h == H - 1))
                nc.tensor.matmul(po[:, 512:832],
                                 outTn2[0:DH, h, t * P:(t + 1) * P],
                                 o2pt[0:DH, h, 320:640],
                                 start=(h == 0), stop=(h == H - 1))
            nc.vector.tensor_add(h3[:, t, 0:320], po[:, 0:320],
                                 h2[:, t, 0:320])
            nc.vector.tensor_add(h3[:, t, 320:640], po[:, 512:832],
                                 h2[:, t, 320:640])
            if has_ob2:
                nc.vector.tensor_add(h3[:, t, :], h3[:, t, :],
                                     bias_tiles["ob2"][:])
        pop2.release()
        o2pp.release()
        outTn2p.release()
        q2Tp.release()

        # ================= FFN (geglu, fp8 DoubleRow) =================
        # x3T8 holds LN3(h3)/8 in fp8 pairs: [:, p, i, :] = kt (2p+i);
        # pair slot (2, 1) is the zero pad for kt=5.
        hgTp = tc.alloc_tile_pool(name="hgTp", bufs=1)
        hgT8 = hgTp.tile([P, NP2, 2, S], fp8, name="hgT8")
        x3p = tc.alloc_tile_pool(name="x3p", bufs=1)
        x3T8 = x3p.tile([P, NP1, 2, S], fp8, name="x3T8")
        nc.vector.memset(x3T8[:, NP1 - 1, 1, :], 0.0)
        ptp3 = tc.alloc_tile_pool(name="ptp3", bufs=2, space=PSUM)
        for t in range(NQT):
            ln_block(h3[:, t, :],
                     lambda kt, t=t: x3T8[:, kt // 2, kt % 2,
                                          t * P:(t + 1) * P],
                     ptp3, rstd_mult=0.125)

        w1pp = tc.alloc_tile_pool(name="w1pp", bufs=8)
        ggp = tc.alloc_tile_pool(name="ggp", bufs=3)
        pw1 = tc.alloc_tile_pool(name="pw1", bufs=6, space=PSUM)
        for mp in range(NM1 // 2):
            wh8 = w1pp.tile([P, NP1, 2, P], fp8, name="wh8", tag="w1")
            nc.sync.dma_start(out=wh8[:], in_=w1p_d[mp])
            wg8 = w1pp.tile([P, NP1, 2, P], fp8, name="wg8", tag="w1")
            nc.sync.dma_start(out=wg8[:], in_=w1p_d[mp + NM1 // 2])
            for qc in range(2):
                ph = pw1.tile([P, 512], f32, name="ph", tag="pw1")
                pg = pw1.tile([P, 512], f32, name="pg", tag="pw1")
                for p in range(NP1):
                    nc.tensor.matmul(
                        ph[:], wh8[:, p, :, :],
                        x3T8[:, p, :, qc * 512:(qc + 1) * 512],
                        start=(p == 0), stop=(p == NP1 - 1),
                        perf_mode=DR)
                for p in range(NP1):
                    nc.tensor.matmul(
                        pg[:], wg8[:, p, :, :],
                        x3T8[:, p, :, qc * 512:(qc + 1) * 512],
                        start=(p == 0), stop=(p == NP1 - 1),
                        perf_mode=DR)
                gg = ggp.tile([P, 512], f32, name="gg", tag="gg")
                if has_b1:
                    nc.scalar.activation(
                        gg[:], pg[:], AF.Gelu_apprx_tanh,
                        bias=bias_tiles["b1p"][:, mp + NM1 // 2:
                                               mp + NM1 // 2 + 1])
                    hb = ggp.tile([P, 512], f32, name="hb", tag="gg")
                    nc.vector.tensor_scalar(
                        out=hb[:], in0=ph[:],
                        scalar1=bias_tiles["b1p"][:, mp:mp + 1],
                        scalar2=4.0, op0=Alu.add, op1=Alu.mult)
                    nc.vector.tensor_mul(
                        hgT8[:, mp // 2, mp % 2, qc * 512:(qc + 1) * 512],
                        hb[:], gg[:])
                else:
                    nc.scalar.activation(gg[:], pg[:], AF.Gelu_apprx_tanh)
                    nc.vector.scalar_tensor_tensor(
                        out=hgT8[:, mp // 2, mp % 2,
                                 qc * 512:(qc + 1) * 512],
                        in0=ph[:], scalar=4.0, in1=gg[:],
                        op0=Alu.mult, op1=Alu.mult)
        pw1.release()
        ggp.release()
        w1pp.release()
        x3p.release()
        ptp3.release()

        pw2 = tc.alloc_tile_pool(name="pw2", bufs=8, space=PSUM)
        for th in range(2):
            pf = []
            for tt in range(4):
                pf.append((pw2.tile([P, 320], f32, name=f"pf{tt}a",
                                    tag="pw2"),
                           pw2.tile([P, 320], f32, name=f"pf{tt}b",
                                    tag="pw2")))
            for pr in range(NP2):
                for tt in range(4):
                    t = th * 4 + tt
                    nc.tensor.matmul(pf[tt][0][:],
                                     hgT8[:, pr, :, t * P:(t + 1) * P],
                                     w2sb[:, pr, :, 0:320],
                                     start=(pr == 0), stop=(pr == NP2 - 1),
                                     perf_mode=DR)
                    nc.tensor.matmul(pf[tt][1][:],
                                     hgT8[:, pr, :, t * P:(t + 1) * P],
                                     w2sb[:, pr, :, 320:640],
                                     start=(pr == 0), stop=(pr == NP2 - 1),
                                     perf_mode=DR)
            for tt in range(4):
                t = th * 4 + tt
                ot = io.tile([P, D], f32, name="ot", tag="io")
                nc.vector.scalar_tensor_tensor(
                    out=ot[:, 0:320], in0=pf[tt][0][:], scalar=1.0 / 64,
                    in1=h3[:, t, 0:320], op0=Alu.mult, op1=Alu.add)
                nc.vector.scalar_tensor_tensor(
                    out=ot[:, 320:640], in0=pf[tt][1][:], scalar=1.0 / 64,
                    in1=h3[:, t, 320:640], op0=Alu.mult, op1=Alu.add)
                if has_fb2:
                    nc.vector.tensor_add(ot[:], ot[:], bias_tiles["fb2"][:])
                nc.sync.dma_start(out=out_d[t * P:(t + 1) * P, :], in_=ot[:])
        pw2.release()
        hgTp.release()

        h3p.release()
        w2sbp.release()
        v2pp.release()
        k2Tp.release()
        h2p.release()
        xop.release()
        io.release()
        stats.release()
        const.release()

    nc.compile()
    return nc


def _prep_inputs(inputs):
    import ml_dtypes

    f32 = np.float32
    bf = ml_dtypes.bfloat16
    f8 = ml_dtypes.float8_e4m3
    g = {k: np.asarray(v) for k, v in inputs.items()}
    hs = np.ascontiguousarray(g["hidden_states"], f32)
    enc = np.ascontiguousarray(g["encoder_hidden_states"], f32)
    f = int(g["video_length"])
    assert hs.shape == (F, S, D) and enc.shape == (F, ENC, CROSS) and f == F

    ln1w, ln1b = g["ln1_w"].astype(f32), g["ln1_b"].astype(f32)
    ln2w, ln2b = g["ln2_w"].astype(f32), g["ln2_b"].astype(f32)
    ln3w, ln3b = g["ln3_w"].astype(f32), g["ln3_b"].astype(f32)
    q1, k1, v1 = (g[n].astype(f32) for n in ("q1", "k1", "v1"))
    o1w, o1b = g["o1_w"].astype(f32), g["o1_b"].astype(f32)
    q2, k2, v2 = (g[n].astype(f32) for n in ("q2", "k2", "v2"))
    o2w, o2b = g["o2_w"].astype(f32), g["o2_b"].astype(f32)
    w1, b1 = g["ff_w1"].astype(f32), g["ff_b1"].astype(f32)
    w2, b2 = g["ff_w2"].astype(f32), g["ff_b2"].astype(f32)

    # ff_w1: fold LN3 scale, multiply by 8 (x3 is pre-scaled by 1/8), pad
    # the 640-dim contraction to 768 and interleave into DoubleRow pairs.
    w1f = (w1 * ln3w[:, None]) * 8.0
    w1pad = np.zeros((768, 2 * FFI), f32)
    w1pad[:D] = w1f
    w1p8 = np.ascontiguousarray(
        w1pad.reshape(NP1, 2, P, NM1, P).transpose(3, 2, 0, 1, 4)).astype(f8)
    # ff_w2: *16 (geglu product carries *4; total 64 undone on output).
    # Layout [P, NP2, 2, D]: partition-major so the whole tensor loads in
    # one contiguous DMA at program start.
    w2p8 = np.ascontiguousarray(
        (w2 * 16.0).reshape(NP2, 2, P, D).transpose(2, 0, 1, 3)).astype(f8)

    shared = {
        "wq1": np.ascontiguousarray(q1 * ln1w[:, None]).astype(bf),
        "wk1": np.ascontiguousarray(k1 * ln1w[:, None]).astype(bf),
        "wv1": np.ascontiguousarray(v1 * ln1w[:, None]).astype(bf),
        "o1p": np.ascontiguousarray(o1w.reshape(H, DH, D)).astype(bf),
        "wq2": np.ascontiguousarray(q2 * ln2w[:, None]).astype(bf),
        "wk2": np.ascontiguousarray(k2),
        "wv2": np.ascontiguousarray(v2),
        "o2p": np.ascontiguousarray(o2w.reshape(H, DH, D)).astype(bf),
        "w1p8": w1p8,
        "w2p8": w2p8,
    }

    qb1 = ln1b @ q1
    kb1 = ln1b @ k1
    vb1 = ln1b @ v1
    q2b = ln2b @ q2
    b1f = b1 + ln3b @ w1
    flags = (
        bool(np.any(qb1)), bool(np.any(kb1)), bool(np.any(vb1)),
        bool(np.any(o1b)), bool(np.any(q2b)), bool(np.any(o2b)),
        bool(np.any(b1f)), bool(np.any(b2)),
    )
    (has_qb1, has_kb1, has_vb1, has_ob1, has_q2b, has_ob2, has_b1,
     has_fb2) = flags
    if has_b1:
        shared["b1p"] = np.ascontiguousarray(b1f.reshape(NM1, P).T)
    if has_qb1:
        shared["qb1"] = np.ascontiguousarray(qb1.reshape(H, DH).T)
    if has_kb1:
        shared["kb1"] = np.ascontiguousarray(kb1.reshape(H, DH).T)
    if has_vb1:
        shared["vb1"] = np.ascontiguousarray(vb1.reshape(H, DH).T)
    if has_q2b:
        shared["q2b"] = np.ascontiguousarray(q2b.reshape(H, DH).T)
    if has_ob1:
        shared["ob1"] = np.ascontiguousarray(np.broadcast_to(o1b, (P, D)))
    if has_ob2:
        shared["ob2"] = np.ascontiguousarray(np.broadcast_to(o2b, (P, D)))
    if has_fb2:
        shared["fb2"] = np.ascontiguousarray(np.broadcast_to(b2, (P, D)))

    former = [0] + list(range(F - 1))
    in_maps = []
    for i in range(F):
        m = dict(shared)
        m["hs3"] = np.ascontiguousarray(
            np.concatenate([hs[i], hs[0], hs[former[i]]], axis=0))
        m["enc"] = np.ascontiguousarray(enc[i])
        in_maps.append(m)
    return flags, in_maps


def get_program(flags):
    if flags not in _PROGRAM_CACHE:
        _PROGRAM_CACHE[flags] = _build_program(flags)
    return _PROGRAM_CACHE[flags]


def run(inputs, trace=False):
    from concourse.bass_utils import run_bass_kernel_spmd

    flags, in_maps = _prep_inputs(inputs)
    nc = get_program(flags)
    res = run_bass_kernel_spmd(nc, in_maps, core_ids=list(range(F)),
                               trace=trace)
    out = np.stack([r["out"] for r in res.results], axis=0)
    return out.astype(np.float32), res


def kernel(**inputs):
    out, _ = run(inputs, trace=False)
    return out


# revision 65
# speedup vs baseline: 1.0417x; 1.0213x over previous
"""Trainium2 Bass kernel for a sparse-causal-attention BasicTransformerBlock.

Sharding: pure data-parallel over the 8 video frames (batch=1, f=8) -- one
frame per NeuronCore, zero collectives.  Each core receives its own frame
plus frame 0 and the previous frame (the sparse-causal KV sources) and
recomputes LN1 + K/V projections for those locally.

v2 design (vs the first working version):
  - every matmul operand is bf16 (fp8 for the FFN) -> half-cost LDWEIGHTS,
    half weight DMA traffic; PSUM accumulation stays fp32.
  - softmax normalization: denominator rows are stacked into a [8, q] tile,
    one batched DVE reciprocal per q-half, broadcast across partitions with
    a K=8 one-hot PE matmul (no DRAM round-trips, no per-head reciprocal).
  - encoder-side attn2 work (enc transpose, K2/V2 projections) hoisted to
    the program start to fill the PE while hs3 streams in.
  - FFN runs fp8e4 DoubleRow matmuls (2 contraction rows/cycle).  To dodge
    fp8 denormals the LN3 output is pre-scaled by 1/8 and w1 by 8 (exact
    cancellation); the geglu product is scaled by 4 and w2 by 16, undone
    with a single *1/64 in the final residual add.
"""

import numpy as np

P = 128
S = 1024          # tokens per frame
D = 640
H = 8
DH = 80
KV = 2 * S        # sparse-causal kv tokens (first frame + prev frame)
ENC = 77
CROSS = 768
FFI = 2560
NQT = S // P      # 8 token tiles
NKT = KV // P     # 16 kv token tiles
ND = D // P       # 5 dim tiles
NE = CROSS // P   # 6 encoder-dim tiles
NM1 = 2 * FFI // P  # 40 ff_w1 out tiles
NP1 = 3             # ff_w1 contraction pairs (640 -> padded 768 = 3*256)
NP2 = FFI // 256    # 10 ff_w2 contraction pairs
SCALE = DH ** -0.5
EPS = 1e-5
F = 8

_PROGRAM_CACHE = {}


def _build_program(flags):
    import concourse.bass as bass
    import concourse.tile as tile
    from concourse import bacc, mybir
    from concourse.masks import make_identity

    f32 = mybir.dt.float32
    f32r = mybir.dt.float32r
    bf16 = mybir.dt.bfloat16
    fp8 = mybir.dt.float8e4
    AF = mybir.ActivationFunctionType
    Alu = mybir.AluOpType
    DR = mybir.MatmulPerfMode.DoubleRow
    PSUM = bass.MemorySpace.PSUM

    (has_qb1, has_kb1, has_vb1, has_ob1, has_q2b, has_ob2, has_b1,
     has_fb2) = flags

    nc = bacc.Bacc(None, target_bir_lowering=False)

    hs3_d = nc.dram_tensor("hs3", [3 * S, D], f32, kind="ExternalInput")[:]
    enc_d = nc.dram_tensor("enc", [ENC, CROSS], f32, kind="ExternalInput")[:]
    wq1_d = nc.dram_tensor("wq1", [D, D], bf16, kind="ExternalInput")[:]
    wk1_d = nc.dram_tensor("wk1", [D, D], bf16, kind="ExternalInput")[:]
    wv1_d = nc.dram_tensor("wv1", [D, D], bf16, kind="ExternalInput")[:]
    o1p_d = nc.dram_tensor("o1p", [H, DH, D], bf16, kind="ExternalInput")[:]
    wq2_d = nc.dram_tensor("wq2", [D, D], bf16, kind="ExternalInput")[:]
    wk2_d = nc.dram_tensor("wk2", [CROSS, D], f32r, kind="ExternalInput")[:]
    wv2_d = nc.dram_tensor("wv2", [CROSS, D], f32r, kind="ExternalInput")[:]
    o2p_d = nc.dram_tensor("o2p", [H, DH, D], bf16, kind="ExternalInput")[:]
    w1p_d = nc.dram_tensor("w1p8", [NM1, P, NP1, 2, P], fp8,
                           kind="ExternalInput")[:]
    w2p_d = nc.dram_tensor("w2p8", [P, NP2, 2, D], fp8,
                           kind="ExternalInput")[:]
    out_d = nc.dram_tensor("out", [S, D], f32, kind="ExternalOutput")[:]

    b1p_d = qb1_d = kb1_d = vb1_d = q2b_d = None
    ob1_d = ob2_d = fb2_d = None
    if has_b1:
        b1p_d = nc.dram_tensor("b1p", [P, NM1], f32, kind="ExternalInput")[:]
    if has_qb1:
        qb1_d = nc.dram_tensor("qb1", [DH, H], f32, kind="ExternalInput")[:]
    if has_kb1:
        kb1_d = nc.dram_tensor("kb1", [DH, H], f32, kind="ExternalInput")[:]
    if has_vb1:
        vb1_d = nc.dram_tensor("vb1", [DH, H], f32, kind="ExternalInput")[:]
    if has_q2b:
        q2b_d = nc.dram_tensor("q2b", [DH, H], f32, kind="ExternalInput")[:]
    if has_ob1:
        ob1_d = nc.dram_tensor("ob1", [P, D], f32, kind="ExternalInput")[:]
    if has_ob2:
        ob2_d = nc.dram_tensor("ob2", [P, D], f32, kind="ExternalInput")[:]
    if has_fb2:
        fb2_d = nc.dram_tensor("fb2", [P, D], f32, kind="ExternalInput")[:]

    with tile.TileContext(nc) as tc:
        # ---------- whole-kernel constants ----------
        const = tc.alloc_tile_pool(name="const", bufs=1)
        idb = const.tile([P, P], bf16, name="idb")
        make_identity(nc, idb)
        idf = const.tile([P, P], f32, name="idf")
        make_identity(nc, idf)
        epst = const.tile([P, 1], f32, name="epst")
        nc.vector.memset(epst[:], EPS)
        ones_f32 = const.tile([P, DH], f32, name="ones_f32")
        nc.vector.memset(ones_f32[:], 1.0)
        onest = const.tile([1, DH], f32r, name="onest")
        nc.vector.tensor_copy(onest[:], ones_f32[0:1, :])
        # one-hot rows for the denominator broadcast: sel[:, h, :] has row h
        # all-ones (out = recip[h] broadcast over DH partitions).  Built with
        # affine_select (block identity): keep 0 where (x - h) != 0 else 1.
        ones_sel = const.tile([8, H, DH], f32r, name="ones_sel")
        ones_scr = const.tile([8, H, DH], f32, name="ones_scr")
        nc.gpsimd.memset(ones_scr[:], 0.0)
        nc.gpsimd.affine_select(
            out=ones_scr[:], in_=ones_scr[:],
            compare_op=Alu.not_equal, fill=1.0, base=0,
            pattern=[[-1, H], [0, DH]], channel_multiplier=1)
        nc.vector.tensor_copy(ones_sel[:], ones_scr[:])
        bias_tiles = {}
        if has_b1:
            t = const.tile([P, NM1], f32, name="b1pt")
            nc.sync.dma_start(out=t[:], in_=b1p_d)
            bias_tiles["b1p"] = t
        for nm, dref in (("qb1", qb1_d), ("kb1", kb1_d), ("vb1", vb1_d),
                         ("q2b", q2b_d)):
            if dref is not None:
                t = const.tile([DH, H], f32, name=nm + "t")
                nc.sync.dma_start(out=t[:], in_=dref)
                bias_tiles[nm] = t
        for nm, dref in (("ob1", ob1_d), ("ob2", ob2_d), ("fb2", fb2_d)):
            if dref is not None:
                t = const.tile([P, D], f32, name=nm + "t")
                nc.sync.dma_start(out=t[:], in_=dref)
                bias_tiles[nm] = t

        stats = tc.alloc_tile_pool(name="stats", bufs=6)
        io = tc.alloc_tile_pool(name="io", bufs=3)
        xop = tc.alloc_tile_pool(name="xop", bufs=3)
        h2p = tc.alloc_tile_pool(name="h2p", bufs=1)
        h2 = h2p.tile([P, NQT, D], f32, name="h2")

        def ln_block(xin, xT_dst, pt_pool, rstd_mult=None):
            """LayerNorm (scale/bias folded into consuming weights) and
            per-128-block PE transpose of the bf16 output.

            xin [128, 640] f32 sbuf; xT_dst(kt) -> [128,128] bf16 dst AP."""
            st = stats.tile([P, 2, 6], f32, name="st", tag="st")
            nc.vector.bn_stats(st[:, 0, :], xin[:, 0:320])
            nc.vector.bn_stats(st[:, 1, :], xin[:, 320:640])
            mv = stats.tile([P, 2], f32, name="mv", tag="mv")
            nc.vector.bn_aggr(mv[:], st[:])
            rstd = stats.tile([P, 1], f32, name="rstd", tag="rstd")
            nc.scalar.activation(rstd[:], mv[:, 1:2], AF.Sqrt, bias=epst[:])
            nc.vector.reciprocal(rstd[:], rstd[:])
            if rstd_mult is not None:
                nc.vector.tensor_scalar(
                    out=rstd[:], in0=rstd[:], scalar1=rstd_mult,
                    scalar2=None, op0=Alu.mult, op1=Alu.bypass)
            mb = stats.tile([P, 1], f32, name="mb", tag="mb")
            nc.vector.tensor_scalar(
                out=mb[:], in0=mv[:, 0:1], scalar1=rstd[:], scalar2=-1.0,
                op0=Alu.mult, op1=Alu.mult)
            xo = xop.tile([P, D], bf16, name="xo", tag="xo")
            nc.scalar.activation(xo[:], xin, AF.Identity,
                                 scale=rstd[:], bias=mb[:])
            for kt in range(ND):
                ptile = pt_pool.tile([P, P], bf16, name="ptile", tag="pt")
                nc.tensor.transpose(ptile[:], xo[:, kt * P:(kt + 1) * P],
                                    idb[:])
                if kt % 2 == 0:
                    nc.scalar.copy(out=xT_dst(kt), in_=ptile[:])
                else:
                    nc.vector.tensor_copy(xT_dst(kt), ptile[:])

        # ================= encoder-side attn2 prep (fills PE early) ======
        k2Tp = tc.alloc_tile_pool(name="k2Tp", bufs=1)
        k2T = k2Tp.tile([P, H, ENC], bf16, name="k2T")
        v2pp = tc.alloc_tile_pool(name="v2pp", bufs=1)
        v2p = v2pp.tile([ENC, H * (DH + 1)], f32r, name="v2p")
        encTp = tc.alloc_tile_pool(name="encTp", bufs=1)
        # fp32r matmuls need an even moving-dim size; pad 77 -> 78 zeros
        encT = encTp.tile([P, NE, ENC + 1], f32r, name="encT")
        zcol = encTp.tile([P, NE], f32, name="zcol")
        nc.vector.memset(zcol[:], 0.0)
        nc.vector.tensor_copy(
            encT[:, :, ENC:ENC + 1],
            zcol[:].rearrange("p (a b) -> p a b", b=1))
        encp = tc.alloc_tile_pool(name="encp", bufs=1)
        enc_sb = encp.tile([ENC, CROSS], f32, name="enc_sb")
        nc.sync.dma_start(out=enc_sb[:], in_=enc_d)
        ptpe = tc.alloc_tile_pool(name="ptpe", bufs=2, space=PSUM)
        for kt in range(NE):
            ptile = ptpe.tile([P, P], f32, name="ptileE", tag="pt")
            nc.tensor.transpose(ptile[0:P, 0:ENC],
                                enc_sb[:, kt * P:(kt + 1) * P],
                                idf[0:ENC, 0:ENC])
            nc.vector.tensor_copy(encT[:, kt, 0:ENC], ptile[0:P, 0:ENC])

        wk2p = tc.alloc_tile_pool(name="wk2p", bufs=6)
        pje = tc.alloc_tile_pool(name="pje", bufs=2, space=PSUM)
        wk2 = []
        for kt in range(NE):
            w = wk2p.tile([P, D], f32r, name=f"wk2{kt}", tag="w")
            nc.sync.dma_start(out=w[:], in_=wk2_d[kt * P:(kt + 1) * P, :])
            wk2.append(w)
        for h in range(H):
            pk2 = pje.tile([P, P], f32, name="pk2", tag="pj")
            for kt in range(NE):
                nc.tensor.matmul(pk2[0:DH, 0:ENC + 1],
                                 wk2[kt][:, h * DH:(h + 1) * DH],
                                 encT[:, kt, :],
                                 start=(kt == 0), stop=(kt == NE - 1))
            nc.scalar.copy(out=k2T[0:DH, h, :], in_=pk2[0:DH, 0:ENC])
        wv2 = []
        for kt in range(NE):
            w = wk2p.tile([P, D], f32r, name=f"wv2{kt}", tag="w")
            nc.sync.dma_start(out=w[:], in_=wv2_d[kt * P:(kt + 1) * P, :])
            wv2.append(w)
        pv20 = pje.tile([P, 320], f32, name="pv20", tag="pj")
        pv21 = pje.tile([P, 320], f32, name="pv21", tag="pj")
        for kt in range(NE):
            nc.tensor.matmul(pv20[0:ENC, :], encT[:, kt, 0:ENC],
                             wv2[kt][:, 0:320],
                             start=(kt == 0), stop=(kt == NE - 1))
            nc.tensor.matmul(pv21[0:ENC, :], encT[:, kt, 0:ENC],
                             wv2[kt][:, 320:640],
                             start=(kt == 0), stop=(kt == NE - 1))
        v2sl = v2p[:].rearrange("p (a b) -> p a b", b=DH + 1)
        nc.vector.tensor_copy(v2sl[:, 0:4, 0:DH],
                              pv20[0:ENC, :].rearrange("p (a b) -> p a b",
                                                       b=DH))
        nc.vector.tensor_copy(v2sl[:, 4:8, 0:DH],
                              pv21[0:ENC, :].rearrange("p (a b) -> p a b",
                                                       b=DH))
        nc.vector.tensor_copy(
            v2sl[:, :, DH:DH + 1],
            ones_f32[0:ENC, 0:H].rearrange("p (a b) -> p a b", b=1))
        wk2p.release()
        pje.release()
        ptpe.release()
        encp.release()
        encTp.release()

        # ================= LN1 + QKV projections =================
        w2sbp = tc.alloc_tile_pool(name="w2sbp", bufs=1)
        w2sb = w2sbp.tile([P, NP2, 2, D], fp8, name="w2sb")
        nc.sync.dma_start(out=w2sb[:], in_=w2p_d)
        kTp = tc.alloc_tile_pool(name="kTp", bufs=1)
        kT = kTp.tile([P, H, KV], bf16, name="kT")
        vpp = tc.alloc_tile_pool(name="vpp", bufs=1)
        vp = vpp.tile([P, NKT, H * (DH + 1)], bf16, name="vp")
        qTp = tc.alloc_tile_pool(name="qTp", bufs=1)
        qT = qTp.tile([P, H, S], bf16, name="qT")

        xowp = tc.alloc_tile_pool(name="xowp", bufs=1)
        xowT = xowp.tile([P, ND, S], bf16, name="xowT")
        xkvp = tc.alloc_tile_pool(name="xkvp", bufs=1)
        xkvT = xkvp.tile([P, ND, KV], bf16, name="xkvT")
        ptp = tc.alloc_tile_pool(name="ptp", bufs=2, space=PSUM)
        pjp = tc.alloc_tile_pool(name="pjp", bufs=2, space=PSUM)
        pvp = tc.alloc_tile_pool(name="pvp", bufs=2, space=PSUM)
        wkvp = tc.alloc_tile_pool(name="wkvp", bufs=6)

        # own frame (rows 0:1024) -> LN -> xowT
        for t in range(NQT):
            xt = io.tile([P, D], f32, name="xt", tag="io")
            nc.sync.dma_start(out=xt[:], in_=hs3_d[t * P:(t + 1) * P, :])
            ln_block(xt[:],
                     lambda kt, t=t: xowT[:, kt, t * P:(t + 1) * P], ptp)
        # kv frames (rows 1024:3072) -> LN -> xkvT
        for t in range(NKT):
            xt = io.tile([P, D], f32, name="xt", tag="io")
            nc.sync.dma_start(out=xt[:],
                              in_=hs3_d[(NQT + t) * P:(NQT + t + 1) * P, :])
            ln_block(xt[:],
                     lambda kt, t=t: xkvT[:, kt, t * P:(t + 1) * P], ptp)

        # Q projection (per head; moving = xowT chunks)
        wq = []
        for kt in range(ND):
            w = wkvp.tile([P, D], bf16, name=f"wq{kt}", tag="w")
            nc.sync.dma_start(out=w[:], in_=wq1_d[kt * P:(kt + 1) * P, :])
            wq.append(w)
        for h in range(H):
            pq = pjp.tile([P, 1024], f32, name="pq", tag="pj")
            for c in range(2):
                for kt in range(ND):
                    nc.tensor.matmul(
                        pq[0:DH, c * 512:(c + 1) * 512],
                        wq[kt][:, h * DH:(h + 1) * DH],
                        xowT[:, kt, c * 512:(c + 1) * 512],
                        start=(kt == 0), stop=(kt == ND - 1))
            if has_qb1:
                nc.vector.tensor_scalar_add(
                    pq[0:DH, :], pq[0:DH, :], bias_tiles["qb1"][:, h:h + 1])
            if h % 2 == 0:
                nc.scalar.copy(out=qT[0:DH, h, :], in_=pq[0:DH, :])
            else:
                nc.vector.tensor_copy(qT[0:DH, h, :], pq[0:DH, :])

        # V projection (stationary = xkvT tiles, moving = wv)
        wv = []
        for kt in range(ND):
            w = wkvp.tile([P, D], bf16, name=f"wv{kt}", tag="w")
            nc.sync.dma_start(out=w[:], in_=wv1_d[kt * P:(kt + 1) * P, :])
            wv.append(w)
        for m in range(NKT):
            pv0 = pvp.tile([P, 320], f32, name="pv0", tag="pv")
            pv1 = pvp.tile([P, 320], f32, name="pv1", tag="pv")
            for kt in range(ND):
                nc.tensor.matmul(pv0[:], xkvT[:, kt, m * P:(m + 1) * P],
                                 wv[kt][:, 0:320],
                                 start=(kt == 0), stop=(kt == ND - 1))
                nc.tensor.matmul(pv1[:], xkvT[:, kt, m * P:(m + 1) * P],
                                 wv[kt][:, 320:640],
                                 start=(kt == 0), stop=(kt == ND - 1))
            vsl = vp[:, m, :].rearrange("p (a b) -> p a b", b=DH + 1)
            nc.vector.tensor_copy(
                vsl[:, 0:4, 0:DH], pv0[:].rearrange("p (a b) -> p a b", b=DH))
            nc.vector.tensor_copy(
                vsl[:, 4:8, 0:DH], pv1[:].rearrange("p (a b) -> p a b", b=DH))
            nc.vector.memset(vsl[:, :, DH:DH + 1], 1.0)

        wk = []
        for kt in range(ND):
            w = wkvp.tile([P, D], bf16, name=f"wk{kt}", tag="w")
            nc.sync.dma_start(out=w[:], in_=wk1_d[kt * P:(kt + 1) * P, :])
            wk.append(w)
        pvp.release()
        pjp.release()
        ptp.release()

        # ======== attn1: K-proj fused into the per-head attention loop ====
        # Head h's attention starts as soon as its own K tiles are done,
        # instead of waiting for all 8 heads' projections.
        outTnp = tc.alloc_tile_pool(name="outTnp", bufs=1)
        outTn = outTnp.tile([P, H, S], bf16, name="outTn")
        o1pp = tc.alloc_tile_pool(name="o1pp", bufs=1)
        o1pt = o1pp.tile([P, H, D], bf16, name="o1pt")
        for h in range(H):
            nc.sync.dma_start(out=o1pt[0:DH, h, :], in_=o1p_d[h])
        expp = tc.alloc_tile_pool(name="expp", bufs=4)
        dstkp = tc.alloc_tile_pool(name="dstkp", bufs=4)
        psp = tc.alloc_tile_pool(name="psp", bufs=2, space=PSUM)
        pavp = tc.alloc_tile_pool(name="pavp", bufs=2, space=PSUM)
        popb = tc.alloc_tile_pool(name="popb", bufs=2, space=PSUM)

        dstacks = [dstkp.tile([8, 512], bf16, name=f"dstack{qc}", tag="dst")
                   for qc in range(2)]
        rstacks = [dstkp.tile([8, 512], f32r, name=f"rstack{qc}", tag="rst")
                   for qc in range(2)]
        for h in range(H):
            # K projection for this head.  PSUM comes from the popb pool
            # (idle until the normalization tail) so K-proj copies never
            # starve the scores pipeline of ps slots.
            for c4 in range(4):
                pk = popb.tile([P, 512], f32, name="pk", tag="popb")
                for kt in range(ND):
                    nc.tensor.matmul(
                        pk[0:DH, :],
                        wk[kt][:, h * DH:(h + 1) * DH],
                        xkvT[:, kt, c4 * 512:(c4 + 1) * 512],
                        start=(kt == 0), stop=(kt == ND - 1))
                if has_kb1:
                    nc.vector.tensor_scalar_add(
                        pk[0:DH, :], pk[0:DH, :],
                        bias_tiles["kb1"][:, h:h + 1])
                nc.vector.tensor_copy(
                    kT[0:DH, h, c4 * 512:(c4 + 1) * 512], pk[0:DH, :])
            for qc in range(2):
                pav = pavp.tile([P, 512], f32, name="pav", tag="pav")
                for kp in range(NKT // 2):
                    ps = psp.tile([P, 1024], f32, name="ps", tag="ps")
                    for half in range(2):
                        kvt = 2 * kp + half
                        nc.tensor.matmul(
                            ps[:, half * 512:(half + 1) * 512],
                            kT[0:DH, h, kvt * P:(kvt + 1) * P],
                            qT[0:DH, h, qc * 512:(qc + 1) * 512],
                            start=True, stop=True)
                    ex = expp.tile([P, 1024], bf16, name="ex", tag="exp")
                    nc.scalar.activation(ex[:], ps[:], AF.Exp, scale=SCALE)
                    for half in range(2):
                        kvt = 2 * kp + half
                        nc.tensor.matmul(
                            pav[0:DH + 1, :],
                            vp[:, kvt, h * (DH + 1):(h + 1) * (DH + 1)],
                            ex[:, half * 512:(half + 1) * 512],
                            start=(kvt == 0), stop=(kvt == NKT - 1))
                qs = slice(qc * 512, (qc + 1) * 512)
                nc.vector.tensor_copy(outTn[0:DH + 1, h, qs],
                                      pav[0:DH + 1, :])
                nc.gpsimd.dma_start(out=dstacks[qc][h:h + 1, :],
                                    in_=outTn[DH:DH + 1, h, qs])
        for qc in range(2):
            with nc.allow_low_precision(reason="f32r softmax denom recip"):
                nc.vector.reciprocal(rstacks[qc][:], dstacks[qc][:])
            for h in range(H):
                qs = slice(qc * 512, (qc + 1) * 512)
                pb = popb.tile([P, 512], f32, name="pb", tag="popb")
                nc.tensor.matmul(pb[0:DH, :], ones_sel[0:8, h, :],
                                 rstacks[qc][:], start=True, stop=True)
                nc.vector.tensor_mul(outTn[0:DH, h, qs],
                                     outTn[0:DH, h, qs], pb[0:DH, :])
                if has_vb1:
                    nc.vector.tensor_scalar_add(
                        outTn[0:DH, h, qc * 512:(qc + 1) * 512],
                        outTn[0:DH, h, qc * 512:(qc + 1) * 512],
                        bias_tiles["vb1"][:, h:h + 1])
        # o1 projection + residual
        for t in range(NQT):
            hres = io.tile([P, D], f32, name="hres", tag="io")
            nc.sync.dma_start(out=hres[:], in_=hs3_d[t * P:(t + 1) * P, :])
            # o1 accumulators live in the (now idle) ps pool as bank-aligned
            # halves of one [P,1024] tile -> 2 tokens in flight keeps the PE
            # dense enough that the HAM clock gate stays at full rate.
            po = psp.tile([P, 1024], f32, name="po", tag="ps")
            for h in range(H):
                nc.tensor.matmul(po[:, 0:320],
                                 outTn[0:DH, h, t * P:(t + 1) * P],
                                 o1pt[0:DH, h, 0:320],
                                 start=(h == 0), stop=(h == H - 1))
                nc.tensor.matmul(po[:, 512:832],
                                 outTn[0:DH, h, t * P:(t + 1) * P],
                                 o1pt[0:DH, h, 320:640],
                                 start=(h == 0), stop=(h == H - 1))
            nc.vector.tensor_add(h2[:, t, 0:320], po[:, 0:320],
                                 hres[:, 0:320])
            nc.vector.tensor_add(h2[:, t, 320:640], po[:, 512:832],
                                 hres[:, 320:640])
            if has_ob1:
                nc.vector.tensor_add(h2[:, t, :], h2[:, t, :],
                                     bias_tiles["ob1"][:])
        popb.release()
        pavp.release()
        psp.release()
        dstkp.release()
        expp.release()
        o1pp.release()
        outTnp.release()
        wkvp.release()
        xkvp.release()
        xowp.release()
        qTp.release()
        vpp.release()
        kTp.release()

        # ================= attn2: cross attention =================
        h3p = tc.alloc_tile_pool(name="h3p", bufs=1)
        h3 = h3p.tile([P, NQT, D], f32, name="h3")
        q2Tp = tc.alloc_tile_pool(name="q2Tp", bufs=1)
        q2T = q2Tp.tile([P, H, S], bf16, name="q2T")
        x2p = tc.alloc_tile_pool(name="x2p", bufs=1)
        x2T = x2p.tile([P, ND, S], bf16, name="x2T")
        ptp2 = tc.alloc_tile_pool(name="ptp2", bufs=2, space=PSUM)
        pjp2 = tc.alloc_tile_pool(name="pjp2", bufs=1, space=PSUM)
        pav2p = tc.alloc_tile_pool(name="pav2p", bufs=2, space=PSUM)
        for t in range(NQT):
            ln_block(h2[:, t, :],
                     lambda kt, t=t: x2T[:, kt, t * P:(t + 1) * P], ptp2)

        wq2p = tc.alloc_tile_pool(name="wq2p", bufs=5)
        wq2 = []
        for kt in range(ND):
            w = wq2p.tile([P, D], bf16, name=f"wq2{kt}", tag="w")
            nc.sync.dma_start(out=w[:], in_=wq2_d[kt * P:(kt + 1) * P, :])
            wq2.append(w)
        for h in range(H):
            # pav2p is idle until scores start -> borrow it so q2 heads
            # double-buffer instead of serializing on the single pjp2 slot.
            pq = pav2p.tile([P, 1024], f32, name="pq2", tag="pav2")
            for c in range(2):
                for kt in range(ND):
                    nc.tensor.matmul(
                        pq[0:DH, c * 512:(c + 1) * 512],
                        wq2[kt][:, h * DH:(h + 1) * DH],
                        x2T[:, kt, c * 512:(c + 1) * 512],
                        start=(kt == 0), stop=(kt == ND - 1))
            if has_q2b:
                nc.vector.tensor_scalar_add(
                    pq[0:DH, :], pq[0:DH, :], bias_tiles["q2b"][:, h:h + 1])
            if h % 2 == 0:
                nc.scalar.copy(out=q2T[0:DH, h, :], in_=pq[0:DH, :])
            else:
                nc.vector.tensor_copy(q2T[0:DH, h, :], pq[0:DH, :])
        wq2p.release()
        x2p.release()

        outTn2p = tc.alloc_tile_pool(name="outTn2p", bufs=1)
        outTn2 = outTn2p.tile([P, H, S], bf16, name="outTn2")
        exp2p = tc.alloc_tile_pool(name="exp2p", bufs=3)
        dstk2p = tc.alloc_tile_pool(name="dstk2p", bufs=1)
        dstack2 = dstk2p.tile([8, 1024], bf16, name="dstack2")
        rstack2 = dstk2p.tile([8, 1024], f32r, name="rstack2")
        for h in range(H):
            ps2 = pjp2.tile([P, 1024], f32, name="ps2", tag="pj2")
            for c in range(2):
                nc.tensor.matmul(ps2[0:ENC, c * 512:(c + 1) * 512],
                                 k2T[0:DH, h, :],
                                 q2T[0:DH, h, c * 512:(c + 1) * 512],
                                 start=True, stop=True)
            ex2 = exp2p.tile([P, 1024], f32r, name="ex2", tag="exp2")
            nc.scalar.activation(ex2[0:ENC, :], ps2[0:ENC, :], AF.Exp,
                                 scale=SCALE)
            pav2 = pav2p.tile([P, 1024], f32, name="pav2", tag="pav2")
            for c in range(2):
                nc.tensor.matmul(pav2[0:DH + 1, c * 512:(c + 1) * 512],
                                 v2p[:, h * (DH + 1):(h + 1) * (DH + 1)],
                                 ex2[0:ENC, c * 512:(c + 1) * 512],
                                 start=True, stop=True)
            nc.vector.tensor_copy(outTn2[0:DH + 1, h, :], pav2[0:DH + 1, :])
            nc.gpsimd.dma_start(out=dstack2[h:h + 1, :],
                                in_=outTn2[DH:DH + 1, h, :])
        with nc.allow_low_precision(reason="f32r softmax denom recip"):
            nc.vector.reciprocal(rstack2[:], dstack2[:])
        for h in range(H):
            pb2 = pav2p.tile([P, 1024], f32, name="pb2", tag="pav2")
            for c in range(2):
                nc.tensor.matmul(pb2[0:DH, c * 512:(c + 1) * 512],
                                 ones_sel[0:8, h, :],
                                 rstack2[:, c * 512:(c + 1) * 512],
                                 start=True, stop=True)
            nc.vector.tensor_mul(outTn2[0:DH, h, :], outTn2[0:DH, h, :],
                                 pb2[0:DH, :])
        pav2p.release()
        pjp2.release()
        ptp2.release()
        dstk2p.release()
        exp2p.release()

        # o2 projection + residual -> h3
        o2pp = tc.alloc_tile_pool(name="o2pp", bufs=1)
        o2pt = o2pp.tile([P, H, D], bf16, name="o2pt")
        for h in range(H):
            nc.sync.dma_start(out=o2pt[0:DH, h, :], in_=o2p_d[h])
        pop2 = tc.alloc_tile_pool(name="pop2", bufs=4, space=PSUM)
        for t in range(NQT):
            po = pop2.tile([P, 1024], f32, name="po2", tag="po2")
            for h in range(H):
                nc.tensor.matmul(po[:, 0:320],
                                 outTn2[0:DH, h, t * P:(t + 1) * P],
                                 o2pt[0:DH, h, 0:320],
                                 start=(h == 0), stop=(h == H - 1))
                nc.tensor.matmul(po[:, 512:832],
                                 outTn2[0:DH, h, t * P:(t + 1) * P],
                                 o2pt[0:DH, h, 320:640],
                                 start=(h == 0), stop=(h == H - 1))
            nc.vector.tensor_add(h3[:, t, 0:320], po[:, 0:320],
                                 h2[:, t, 0:320])
            nc.vector.tensor_add(h3[:, t, 320:640], po[:, 512:832],
                                 h2[:, t, 320:640])
            if has_ob2:
                nc.vector.tensor_add(h3[:, t, :], h3[:, t, :],
                                     bias_tiles["ob2"][:])
        pop2.release()
        o2pp.release()
        outTn2p.release()
        q2Tp.release()

        # ================= FFN (geglu, fp8 DoubleRow) =================
        # x3T8 holds LN3(h3)/8 in fp8 pairs: [:, p, i, :] = kt (2p+i);
        # pair slot (2, 1) is the zero pad for kt=5.
        hgTp = tc.alloc_tile_pool(name="hgTp", bufs=1)
        hgT8 = hgTp.tile([P, NP2, 2, S], fp8, name="hgT8")
        x3p = tc.alloc_tile_pool(name="x3p", bufs=1)
        x3T8 = x3p.tile([P, NP1, 2, S], fp8, name="x3T8")
        nc.vector.memset(x3T8[:, NP1 - 1, 1, :], 0.0)
        ptp3 = tc.alloc_tile_pool(name="ptp3", bufs=2, space=PSUM)
        for t in range(NQT):
            ln_block(h3[:, t, :],
                     lambda kt, t=t: x3T8[:, kt // 2, kt % 2,
                                          t * P:(t + 1) * P],
                     ptp3, rstd_mult=0.125)

        w1pp = tc.alloc_tile_pool(name="w1pp", bufs=8)
        ggp = tc.alloc_tile_pool(name="ggp", bufs=3)
        pw1 = tc.alloc_tile_pool(name="pw1", bufs=6, space=PSUM)
        for mp in range(NM1 // 2):
            wh8 = w1pp.tile([P, NP1, 2, P], fp8, name="wh8", tag="w1")
            nc.sync.dma_start(out=wh8[:], in_=w1p_d[mp])
            wg8 = w1pp.tile([P, NP1, 2, P], fp8, name="wg8", tag="w1")
            nc.sync.dma_start(out=wg8[:], in_=w1p_d[mp + NM1 // 2])
            for qc in range(2):
                ph = pw1.tile([P, 512], f32, name="ph", tag="pw1")
                pg = pw1.tile([P, 512], f32, name="pg", tag="pw1")
                for p in range(NP1):
                    nc.tensor.matmul(
                        ph[:], wh8[:, p, :, :],
                        x3T8[:, p, :, qc * 512:(qc + 1) * 512],
                        start=(p == 0), stop=(p == NP1 - 1),
                        perf_mode=DR)
                for p in range(NP1):
                    nc.tensor.matmul(
                        pg[:], wg8[:, p, :, :],
                        x3T8[:, p, :, qc * 512:(qc + 1) * 512],
                        start=(p == 0), stop=(p == NP1 - 1),
                        perf_mode=DR)
                gg = ggp.tile([P, 512], f32, name="gg", tag="gg")
                if has_b1:
                    nc.scalar.activation(
                        gg[:], pg[:], AF.Gelu_apprx_tanh,
                        bias=bias_tiles["b1p"][:, mp + NM1 // 2:
                                               mp + NM1 // 2 + 1])
                    hb = ggp.tile([P, 512], f32, name="hb", tag="gg")
                    nc.vector.tensor_scalar(
                        out=hb[:], in0=ph[:],
                        scalar1=bias_tiles["b1p"][:, mp:mp + 1],
                        scalar2=4.0, op0=Alu.add, op1=Alu.mult)
                    nc.vector.tensor_mul(
                        hgT8[:, mp // 2, mp % 2, qc * 512:(qc + 1) * 512],
                        hb[:], gg[:])
                else:
                    nc.scalar.activation(gg[:], pg[:], AF.Gelu_apprx_tanh)
                    nc.vector.scalar_tensor_tensor(
                        out=hgT8[:, mp // 2, mp % 2,
                                 qc * 512:(qc + 1) * 512],
                        in0=ph[:], scalar=4.0, in1=gg[:],
                        op0=Alu.mult, op1=Alu.mult)
        pw1.release()
        ggp.release()
        w1pp.release()
        x3p.release()
        ptp3.release()

        pw2 = tc.alloc_tile_pool(name="pw2", bufs=8, space=PSUM)
        for th in range(2):
            pf = []
            for tt in range(4):
                pf.append((pw2.tile([P, 320], f32, name=f"pf{tt}a",
                                    tag="pw2"),
                           pw2.tile([P, 320], f32, name=f"pf{tt}b",
                                    tag="pw2")))
            for pr in range(NP2):
                for tt in range(4):
                    t = th * 4 + tt
                    nc.tensor.matmul(pf[tt][0][:],
                                     hgT8[:, pr, :, t * P:(t + 1) * P],
                                     w2sb[:, pr, :, 0:320],
                                     start=(pr == 0), stop=(pr == NP2 - 1),
                                     perf_mode=DR)
                    nc.tensor.matmul(pf[tt][1][:],
                                     hgT8[:, pr, :, t * P:(t + 1) * P],
                                     w2sb[:, pr, :, 320:640],
                                     start=(pr == 0), stop=(pr == NP2 - 1),
                                     perf_mode=DR)
            for tt in range(4):
                t = th * 4 + tt
                ot = io.tile([P, D], f32, name="ot", tag="io")
                nc.vector.scalar_tensor_tensor(
                    out=ot[:, 0:320], in0=pf[tt][0][:], scalar=1.0 / 64,
                    in1=h3[:, t, 0:320], op0=Alu.mult, op1=Alu.add)
                nc.vector.scalar_tensor_tensor(
                    out=ot[:, 320:640], in0=pf[tt][1][:], scalar=1.0 / 64,
                    in1=h3[:, t, 320:640], op0=Alu.mult, op1=Alu.add)
                if has_fb2:
                    nc.vector.tensor_add(ot[:], ot[:], bias_tiles["fb2"][:])
                nc.sync.dma_start(out=out_d[t * P:(t + 1) * P, :], in_=ot[:])
        pw2.release()
        hgTp.release()

        h3p.release()
        w2sbp.release()
        v2pp.release()
        k2Tp.release()
        h2p.release()
        xop.release()
        io.release()
        stats.release()
        const.release()

    nc.compile()
    return nc


def _prep_inputs(inputs):
    import ml_dtypes

    f32 = np.float32
    bf = ml_dtypes.bfloat16
    f8 = ml_dtypes.float8_e4m3
    g = {k: np.asarray(v) for k, v in inputs.items()}
    hs = np.ascontiguousarray(g["hidden_states"], f32)
    enc = np.ascontiguousarray(g["encoder_hidden_states"], f32)
    f = int(g["video_length"])
    assert hs.shape == (F, S, D) and enc.shape == (F, ENC, CROSS) and f == F

    ln1w, ln1b = g["ln1_w"].astype(f32), g["ln1_b"].astype(f32)
    ln2w, ln2b = g["ln2_w"].astype(f32), g["ln2_b"].astype(f32)
    ln3w, ln3b = g["ln3_w"].astype(f32), g["ln3_b"].astype(f32)
    q1, k1, v1 = (g[n].astype(f32) for n in ("q1", "k1", "v1"))
    o1w, o1b = g["o1_w"].astype(f32), g["o1_b"].astype(f32)
    q2, k2, v2 = (g[n].astype(f32) for n in ("q2", "k2", "v2"))
    o2w, o2b = g["o2_w"].astype(f32), g["o2_b"].astype(f32)
    w1, b1 = g["ff_w1"].astype(f32), g["ff_b1"].astype(f32)
    w2, b2 = g["ff_w2"].astype(f32), g["ff_b2"].astype(f32)

    # ff_w1: fold LN3 scale, multiply by 8 (x3 is pre-scaled by 1/8), pad
    # the 640-dim contraction to 768 and interleave into DoubleRow pairs.
    w1f = (w1 * ln3w[:, None]) * 8.0
    w1pad = np.zeros((768, 2 * FFI), f32)
    w1pad[:D] = w1f
    w1p8 = np.ascontiguousarray(
        w1pad.reshape(NP1, 2, P, NM1, P).transpose(3, 2, 0, 1, 4)).astype(f8)
    # ff_w2: *16 (geglu product carries *4; total 64 undone on output).
    # Layout [P, NP2, 2, D]: partition-major so the whole tensor loads in
    # one contiguous DMA at program start.
    w2p8 = np.ascontiguousarray(
        (w2 * 16.0).reshape(NP2, 2, P, D).transpose(2, 0, 1, 3)).astype(f8)

    shared = {
        "wq1": np.ascontiguousarray(q1 * ln1w[:, None]).astype(bf),
        "wk1": np.ascontiguousarray(k1 * ln1w[:, None]).astype(bf),
        "wv1": np.ascontiguousarray(v1 * ln1w[:, None]).astype(bf),
        "o1p": np.ascontiguousarray(o1w.reshape(H, DH, D)).astype(bf),
        "wq2": np.ascontiguousarray(q2 * ln2w[:, None]).astype(bf),
        "wk2": np.ascontiguousarray(k2),
        "wv2": np.ascontiguousarray(v2),
        "o2p": np.ascontiguousarray(o2w.reshape(H, DH, D)).astype(bf),
        "w1p8": w1p8,
        "w2p8": w2p8,
    }

    qb1 = ln1b @ q1
    kb1 = ln1b @ k1
    vb1 = ln1b @ v1
    q2b = ln2b @ q2
    b1f = b1 + ln3b @ w1
    flags = (
        bool(np.any(qb1)), bool(np.any(kb1)), bool(np.any(vb1)),
        bool(np.any(o1b)), bool(np.any(q2b)), bool(np.any(o2b)),
        bool(np.any(b1f)), bool(np.any(b2)),
    )
    (has_qb1, has_kb1, has_vb1, has_ob1, has_q2b, has_ob2, has_b1,
     has_fb2) = flags
    if has_b1:
        shared["b1p"] = np.ascontiguousarray(b1f.reshape(NM1, P).T)
    if has_qb1:
        shared["qb1"] = np.ascontiguousarray(qb1.reshape(H, DH).T)
    if has_kb1:
        shared["kb1"] = np.ascontiguousarray(kb1.reshape(H, DH).T)
    if has_vb1:
        shared["vb1"] = np.ascontiguousarray(vb1.reshape(H, DH).T)
    if has_q2b:
        shared["q2b"] = np.ascontiguousarray(q2b.reshape(H, DH).T)
    if has_ob1:
        shared["ob1"] = np.ascontiguousarray(np.broadcast_to(o1b, (P, D)))
    if has_ob2:
        shared["ob2"] = np.ascontiguousarray(np.broadcast_to(o2b, (P, D)))
    if has_fb2:
        shared["fb2"] = np.ascontiguousarray(np.broadcast_to(b2, (P, D)))

    former = [0] + list(range(F - 1))
    in_maps = []
    for i in range(F):
        m = dict(shared)
        m["hs3"] = np.ascontiguousarray(
            np.concatenate([hs[i], hs[0], hs[former[i]]], axis=0))
        m["enc"] = np.ascontiguousarray(enc[i])
        in_maps.append(m)
    return flags, in_maps


def get_program(flags):
    if flags not in _PROGRAM_CACHE:
        _PROGRAM_CACHE[flags] = _build_program(flags)
    return _PROGRAM_CACHE[flags]


def run(inputs, trace=False):
    from concourse.bass_utils import run_bass_kernel_spmd

    flags, in_maps = _prep_inputs(inputs)
    nc = get_program(flags)
    res = run_bass_kernel_spmd(nc, in_maps, core_ids=list(range(F)),
                               trace=trace)
    out = np.stack([r["out"] for r in res.results], axis=0)
    return out.astype(np.float32), res


def kernel(**inputs):
    out, _ = run(inputs, trace=False)
    return out
